# revision 29
# baseline (speedup 1.0000x reference)
"""ViT attention block with relative position bias, SPMD over 8 TRN2 NeuronCores.

Sharding: data-parallel over batch (B=128 -> 16 images per core), weights and
bias table replicated. No collectives.

v8 design (per core, 16 images = 3152 tokens):
  - q/k GEMM in fp8 (e4m3), DoubleRow perf mode, x error-split only
    ((X_hi+X_lo)@W8, 6 passes); v GEMM token-major with the full hi/lo
    split (9 passes), landing in v_aug [tok, 12, 65] (65th col = ones for
    softmax denominators). v bias is folded into the proj bias on the host
    (attn rows sum to 1), so the v evac is a pure scale (DVE/ACT).
  - scores in fp8 DoubleRow: q evacuated as (hi, lo) fp8 pair at scale SQ8
    (near-exact), k single-quantized at SK8; k's qkv bias dropped
    (softmax-invariant per query). 1 DR pass per key tile instead of 2
    bf16 passes. exp on ACT with scale 1/(SQ8*SK8); times exp(bias) on DVE.
  - AV token-major bf16 with denominator column; reciprocal + normalize on
    DVE; PE transposes to feature-major attn_T [128, 6, T] bf16 via a
    dedicated single-bank psum pool (decoupled from the qkv/proj pool).
  - projection bf16 from attn_T; bias add on DVE.
  - all parameter loads issued up front across the SP/ACT/Pool DMA queues;
    x fetched as one contiguous 4800B/partition transfer per pair (token
    dim padded to 400 on the host), prefetched two pairs ahead (3 buffers).
  - attention of pair g-1 interleaves with the qkv m-groups of pair g,
    with AV batches spread between the two half-head score batches; the
    last pair's attention interleaves with early proj tiles, and the late
    proj tiles start as soon as their attn_T columns are transposed.
"""

import sys

import numpy as np

sys.path.insert(0, "/opt/trn_rl_repo")

import ml_dtypes  # noqa: E402

import concourse.bass as bass  # noqa: E402
import concourse.mybir as mybir  # noqa: E402
import concourse.tile as tile  # noqa: E402
from concourse import bacc  # noqa: E402
from concourse.bass_utils import run_bass_kernel_spmd  # noqa: E402
from concourse.masks import make_identity  # noqa: E402

NCORES = 8
B = 128
N = 197
D = 768
H = 12
HD = 64
BL = B // NCORES          # 16 images per core
T = BL * N                # 3152 tokens per core
G = BL // 2               # 8 image pairs
PW = 2 * N                # 394 tokens per pair
KC = D // 128             # 6 contraction chunks
SCALE = HD ** -0.5
SX = 16.0                 # fp8 scale for x
SW = 256.0                # fp8 scale for qkv weights
PS_SCL = 1.0 / (SX * SW)  # psum de-scale
SQ8 = 512.0               # fp8 scale for scaled-q (hi/lo split)
SK8 = 64.0                # fp8 scale for k (single quant)
E_SCL = 1.0 / (SQ8 * SK8)  # descale applied inside the score exp
XSPLIT_QK = True          # q/k GEMM: x split only, w single-quantized
F32 = mybir.dt.float32
BF16 = mybir.dt.bfloat16
FP8 = mybir.dt.float8e4
EXP = mybir.ActivationFunctionType.Exp
IDENT = mybir.ActivationFunctionType.Identity
DR = mybir.MatmulPerfMode.DoubleRow
MUL = mybir.AluOpType.mult
ADD = mybir.AluOpType.add
SUB = mybir.AluOpType.subtract

LAST_EXEC_NS = None
_GRAPH = None


def _bcast_ap(ap_1d, parts):
    """[n] DRAM AP -> [parts, n] AP replicated across partitions."""
    return bass.AP(tensor=ap_1d.tensor, offset=ap_1d.offset,
                   ap=[[0, parts]] + [list(d) for d in ap_1d.ap])


def _free_bcast(ap3, count):
    """[p, h, 1] AP -> [p, h, count] AP with step-0 last dim."""
    dims = [list(d) for d in ap3.ap]
    dims[-1] = [0, count]
    return bass.AP(tensor=ap3.tensor, offset=ap3.offset, ap=dims)


def _dup2(ap2):
    """[p, m] AP -> [p, 2, m] AP with stride-0 k-tile dim (DoubleRow dup)."""
    dims = [list(d) for d in ap2.ap]
    return bass.AP(tensor=ap2.tensor, offset=ap2.offset,
                   ap=[dims[0], [0, 2], dims[1]])


def _build():
    nc = bacc.Bacc("TRN2", target_bir_lowering=False, debug=False,
                   num_devices=NCORES)
    xhl = nc.declare_dram_parameter("xhl", [G, 128, KC, 2, 400], FP8,
                                    isOutput=False)
    w8 = nc.declare_dram_parameter("w8", [128, KC, 2, 3 * D], FP8,
                                   isOutput=False)
    qkb = nc.declare_dram_parameter("qkb", [128, 12], F32, isOutput=False)
    wproj = nc.declare_dram_parameter("wproj", [D, D], BF16, isOutput=False)
    bproj = nc.declare_dram_parameter("bproj", [D], F32, isOutput=False)
    ebias = nc.declare_dram_parameter("ebias", [H, 128, PW], BF16,
                                      isOutput=False)
    out = nc.declare_dram_parameter("out", [T, D], F32, isOutput=True)

    w8r = w8.ap()
    from contextlib import ExitStack
    with tile.TileContext(nc) as tc, ExitStack() as ctx:
        wpool = ctx.enter_context(tc.tile_pool(name="weights", bufs=1))
        xpool = ctx.enter_context(tc.tile_pool(name="xg", bufs=3))
        qkpool = ctx.enter_context(tc.tile_pool(name="qkg", bufs=2))
        qfpool = ctx.enter_context(tc.tile_pool(name="qf", bufs=2))
        vapool = ctx.enter_context(tc.tile_pool(name="vaug", bufs=8))
        ptpool = ctx.enter_context(tc.tile_pool(name="pt", bufs=2))
        atpool = ctx.enter_context(tc.tile_pool(name="at", bufs=4))
        rcpool = ctx.enter_context(tc.tile_pool(name="rcp", bufs=8))
        opool = ctx.enter_context(tc.tile_pool(name="osb", bufs=3))
        ps_big = ctx.enter_context(tc.tile_pool(name="psbig", bufs=2,
                                                space="PSUM"))
        ps_tr = ctx.enter_context(tc.tile_pool(name="pstr", bufs=1,
                                               space="PSUM"))  # transposes
        ps_v = ctx.enter_context(tc.tile_pool(name="psv", bufs=1,
                                              space="PSUM"))  # 1 bank
        ps_sc = ctx.enter_context(tc.tile_pool(name="pssc", bufs=2,
                                               space="PSUM"))
        ps_av = ctx.enter_context(tc.tile_pool(name="psav", bufs=2,
                                               space="PSUM"))

        # ---- persistent weights / constants ----
        # All parameter loads are issued up front, spread over the four DMA
        # queues (SP / Pool / ACT / DVE) so they run concurrently and the
        # pair-0 GEMMs are not starved behind a serial queue.
        w8t = wpool.tile([128, KC, 2, 3 * D], FP8, tag="w8", name="w8")
        qkbt = wpool.tile([128, 12], F32, tag="qkb")
        ident = wpool.tile([128, 128], BF16, tag="ident", name="ident")
        attn_T = wpool.tile([128, KC, T + 16], BF16, tag="attnT",
                            name="attnT")
        eb_all = wpool.tile([128, H, PW], BF16, tag="eball", name="eball")
        w_pj = wpool.tile([128, KC, D], BF16, tag="wproj", name="wproj")
        pjb = wpool.tile([128, D], F32, tag="pjb", name="pjb")

        # ---- qkv steps for one pair ----
        vmap = {}
        qkg_map = {}
        xg_tiles = {}

        def _fetch_xg(g, queue=None):
            # token dim padded to 400 on the host so the transfer is one
            # contiguous 4800B run per partition (and the hi/lo stride is
            # 16B-aligned for the dual-fp8 ldweights restriction)
            xg = xpool.tile([128, KC, 2, 400], FP8, tag="xg",
                            name=f"xg{g}")
            (queue or nc.sync).dma_start(out=xg[:], in_=xhl.ap()[g])
            xg_tiles[g] = xg

        # critical path first: q weights (SP) and x of pair 0 (ACT) in
        # parallel; k (ACT) and v (Pool) land before their first use.
        nc.sync.dma_start(out=w8t[:, :, :, 0:D], in_=w8r[:, :, :, 0:D])
        _fetch_xg(0, queue=nc.scalar)
        _fetch_xg(1)
        nc.scalar.dma_start(out=qkbt[:], in_=qkb.ap())
        nc.gpsimd.dma_start(out=w8t[:, :, :, 2 * D:3 * D],
                            in_=w8r[:, :, :, 2 * D:3 * D])
        nc.scalar.dma_start(out=w8t[:, :, :, D:2 * D],
                            in_=w8r[:, :, :, D:2 * D])
        nc.scalar.dma_start(out=eb_all[:],
                            in_=ebias.ap().rearrange("h p t -> p h t"))
        nc.scalar.dma_start(out=pjb[:], in_=_bcast_ap(bproj.ap()[:], 128))
        nc.gpsimd.dma_start(
            out=w_pj[:],
            in_=wproj.ap().rearrange("(c p) n -> p c n", p=128))
        make_identity(nc, ident[:])

        def qkv_steps(g):
            if g + 2 < G:
                _fetch_xg(g + 2)  # prefetch x two pairs ahead (3 buffers)
            xg = xg_tiles.pop(g)
            qkg = []
            qkg_map[g] = qkg

            def make_qk(m):
                def step():
                    ps = ps_big.tile([128, 512], F32, tag="big")
                    w_m = slice(m * 128, (m + 1) * 128)
                    for c in range(KC):
                        nc.tensor.matmul(ps[:, 0:PW],
                                         _dup2(w8t[:, c, 0, w_m]),
                                         xg[:, c, :, 0:PW],
                                         start=(c == 0),
                                         stop=(XSPLIT_QK and c == KC - 1),
                                         perf_mode=DR, skip_group_check=True)
                    if not XSPLIT_QK:
                        for p in range(KC // 2):
                            nc.tensor.matmul(ps[:, 0:PW],
                                             w8t[:, 2 * p:2 * p + 2, 1, w_m],
                                             xg[:, 2 * p:2 * p + 2, 0, 0:PW],
                                             start=False,
                                             stop=(p == KC // 2 - 1),
                                             perf_mode=DR,
                                             skip_group_check=True)
                    if m < 6:
                        # q: bf16 intermediate (pre-scaled by SQ8, bias in
                        # qkbt already carries SQ8) -> fp8 (hi, lo) pair.
                        # Pool cannot read PSUM and has no TensorScalar, so
                        # qf comes via DVE/ACT; hi is a convert, lo a sub.
                        scl = PS_SCL * SCALE * SQ8
                        qf = qfpool.tile([128, PW], BF16, tag="qf",
                                         name=f"qf{m}_{g}")
                        if m % 2 == 0:
                            nc.vector.tensor_scalar(qf[:], ps[:, 0:PW], scl,
                                                    qkbt[:, m:m + 1],
                                                    MUL, ADD)
                        else:
                            nc.scalar.activation(qf[:], ps[:, 0:PW], IDENT,
                                                 bias=qkbt[:, m:m + 1],
                                                 scale=scl)
                        q8 = qkpool.tile([128, 2, 400], FP8, tag=f"q8{m}",
                                         name=f"q8{m}_{g}")
                        if m % 2 == 0:
                            nc.scalar.activation(q8[:, 0, 0:PW], qf[:],
                                                 IDENT)
                            nc.gpsimd.tensor_sub(q8[:, 1, 0:PW], qf[:],
                                                 q8[:, 0, 0:PW])
                        else:
                            nc.gpsimd.tensor_copy(q8[:, 0, 0:PW], qf[:])
                            nc.vector.tensor_sub(q8[:, 1, 0:PW], qf[:],
                                                 q8[:, 0, 0:PW])
                        qkg.append(q8)
                    else:
                        # k: single fp8 quant, bias dropped (softmax-inv.)
                        k8 = qkpool.tile([128, 400], FP8, tag=f"k8{m}",
                                         name=f"k8{m}_{g}")
                        if m % 2 == 0:
                            nc.scalar.activation(k8[:, 0:PW], ps[:, 0:PW],
                                                 IDENT, scale=PS_SCL * SK8)
                        else:
                            nc.vector.tensor_scalar(k8[:, 0:PW], ps[:, 0:PW],
                                                    PS_SCL * SK8, None, MUL)
                        qkg.append(k8)

                return step

            def make_v(j, gidx):
                def step():
                    img, it = j // 2, j % 2
                    gi = 2 * g + img
                    t0 = img * N + it * 128
                    tsz = 128 if it == 0 else 70  # even M for dual-fp8 LW
                    n0, nsz = (0, 512) if gidx == 0 else (512, 256)
                    ps = ps_v.tile([128, 512], F32, tag="pv")
                    wv = slice(2 * D + n0, 2 * D + n0 + nsz)
                    for c in range(KC):
                        nc.tensor.matmul(
                            ps[0:tsz, 0:nsz],
                            xg[:, c, :, t0:t0 + tsz],
                            _dup2(w8t[:, c, 0, wv]),
                            start=(c == 0), stop=False,
                            perf_mode=DR, skip_group_check=True)
                    for p in range(KC // 2):
                        nc.tensor.matmul(
                            ps[0:tsz, 0:nsz],
                            xg[:, 2 * p:2 * p + 2, 0, t0:t0 + tsz],
                            w8t[:, 2 * p:2 * p + 2, 1, wv],
                            start=False, stop=(p == KC // 2 - 1),
                            perf_mode=DR, skip_group_check=True)
                    if gidx == 0:
                        va = vapool.tile([128, H, HD + 1], BF16,
                                         tag=f"va{it}", name=f"va{it}_{gi}")
                        nc.gpsimd.memset(va[0:tsz, :, HD:HD + 1], 1.0)
                        if it == 0:
                            vmap[gi] = [va, None]
                        else:
                            vmap[gi][1] = va
                    va = vmap[gi][it]
                    hs = slice(0, 8) if gidx == 0 else (slice(8, 12))
                    # v bias is folded into the proj bias on the host, so
                    # the evac is a pure scale; alternate DVE/ACT.
                    ps3 = ps[0:tsz, 0:nsz].rearrange("p (h d) -> p h d",
                                                     d=HD)
                    if (j + gidx) % 2 == 0:
                        nc.vector.tensor_scalar(va[0:tsz, hs, 0:HD], ps3,
                                                PS_SCL, None, MUL)
                    else:
                        nc.scalar.activation(va[0:tsz, hs, 0:HD], ps3,
                                             IDENT, scale=PS_SCL)

                return step

            steps = [make_qk(m) for m in range(12)]
            # spread the 8 v half-tiles between qk m-groups to hide evac
            # latency (after m==1 has kicked the v-weight DMA on pair 0)
            vsteps = [make_v(j, gx) for j in range(4) for gx in range(2)]
            for i, vs in enumerate(reversed(vsteps)):
                steps.insert(12 - i, vs)
            return steps

        # ---- attention steps for one pair ----
        def attn_steps(g):
            qkg = qkg_map.pop(g)
            steps = []
            pts = {}

            def score_step(img, h):
                def step():
                    if h == 0:
                        pts[img] = [
                            ptpool.tile([128, 6, PW], BF16, tag="pt0",
                                        name=f"pt0_{2 * g + img}"),
                            ptpool.tile([128, 6, PW], BF16, tag="pt1",
                                        name=f"pt1_{2 * g + img}")]
                    pt = pts[img][h // 6]
                    co = img * N
                    mq = h // 2
                    ro = (h % 2) * 64
                    mk = 6 + h // 2
                    q8 = qkg[mq]
                    k8 = qkg[mk]
                    ps = ps_sc.tile([128, PW], F32, tag="sc")
                    nc.tensor.matmul(ps[:, 0:N],
                                     _dup2(k8[ro:ro + 64, co:co + 128]),
                                     q8[ro:ro + 64, :, co:co + N],
                                     start=True, stop=True,
                                     perf_mode=DR, skip_group_check=True)
                    nc.tensor.matmul(ps[0:70, N:2 * N],
                                     _dup2(k8[ro:ro + 64, co + 128:co + 198]),
                                     q8[ro:ro + 64, :, co:co + N],
                                     start=True, stop=True,
                                     perf_mode=DR, skip_group_check=True)
                    hh = h % 6
                    nc.scalar.activation(pt[:, hh, :], ps[:], EXP,
                                         scale=E_SCL)
                    # all on DVE: a slow Pool multiply in the batch would
                    # gate the AV matmuls that consume the full pt half
                    nc.vector.tensor_mul(pt[:, hh, :], pt[:, hh, :],
                                         eb_all[:, h, :])

                return step

            def av_half(img, it, half):
                def step():
                    pt = pts[img][half]
                    va0, va1 = vmap[2 * g + img]
                    i0, isz = (0, 128) if it == 0 else (128, 69)
                    if half == 0:
                        ats[(img, it)] = atpool.tile(
                            [128, D], BF16, tag=f"at{it}",
                            name=f"at{it}_{g}_{img}")
                    at = ats[(img, it)]
                    av = ps_av.tile([128, 6 * 65], F32, tag="av")
                    for hh in range(6):
                        h = half * 6 + hh
                        nc.tensor.matmul(av[0:isz, hh * 65:(hh + 1) * 65],
                                         pt[:, hh, i0:i0 + isz],
                                         va0[:, h, :],
                                         start=True, stop=False)
                        nc.tensor.matmul(av[0:isz, hh * 65:(hh + 1) * 65],
                                         pt[0:69, hh, N + i0:N + i0 + isz],
                                         va1[0:69, h, :],
                                         start=False, stop=True)
                    av3 = av[0:isz].rearrange("p (h x) -> p h x", x=65)
                    rc = rcpool.tile([128, 6, 1], F32, tag="rc")
                    nc.vector.reciprocal(rc[0:isz], av3[:, :, 64:65])
                    nc.vector.tensor_mul(
                        at[0:isz, half * 384:(half + 1) * 384]
                        .rearrange("p (h x) -> p h x", x=HD),
                        av3[:, :, 0:HD],
                        _free_bcast(rc[0:isz], HD))

                return step

            def av_tr(img, it):
                def step():
                    at = ats[(img, it)]
                    gcol = g * PW + img * N
                    i0, isz = (0, 128) if it == 0 else (128, 69)
                    tcol = gcol + i0
                    # [128, 768] bf16 = 1536B fits a single psum bank
                    tp = ps_tr.tile([128, KC * 128], BF16, tag="tr")
                    for c in range(KC):
                        nc.tensor.transpose(tp[:, c * 128:c * 128 + isz],
                                            at[0:isz, c * 128:(c + 1) * 128],
                                            ident[0:isz, 0:isz])
                    nc.vector.tensor_copy(
                        attn_T[:, :, tcol:tcol + isz],
                        tp[:].rearrange("p (c t) -> p c t", t=128)
                        [:, :, 0:isz])

                return step

            ats = {}
            # interleave AV between the two half-head score batches so the
            # ACT exp chain (and the eb/normalize work) spreads across the
            # pair instead of clustering at its head
            for img in range(2):
                for h in range(6):
                    steps.append(score_step(img, h))
                if img == 1:
                    steps.append(av_half(0, 0, 1))
                    steps.append(av_half(0, 1, 1))
                    steps.append(av_tr(0, 0))
                    steps.append(av_tr(0, 1))
                for h in range(6, H):
                    steps.append(score_step(img, h))
                steps.append(av_half(img, 0, 0))
                steps.append(av_half(img, 1, 0))
            steps.append(av_half(1, 0, 1))
            steps.append(av_half(1, 1, 1))
            steps.append(av_tr(1, 0))
            steps.append(av_tr(1, 1))
            return steps

        # ---- main loop: interleave qkv(g) with attention(g-1) ----
        pending = []

        def drain(k):
            for _ in range(k):
                if pending:
                    pending.pop(0)()

        for g in range(G):
            qs = qkv_steps(g)
            n_q = len(qs)
            n_a = len(pending)
            for i, q in enumerate(qs):
                want = ((i + 1) * n_a) // n_q
                done = n_a - len(pending)
                drain(want - done)
                q()
            drain(len(pending))
            pending = attn_steps(g)

        # ---- output projection interleaved with last pair's attention ----
        def proj_step(t0):
            def step():
                sz = min(128, T - t0)
                ot = opool.tile([128, D], F32, tag="osb")
                for (n0, nsz) in ((0, 512), (512, 256)):
                    ps = ps_big.tile([128, 512], F32, tag="big")
                    for c in range(KC):
                        nc.tensor.matmul(ps[0:sz, 0:nsz],
                                         attn_T[:, c, t0:t0 + sz],
                                         w_pj[:, c, n0:n0 + nsz],
                                         start=(c == 0), stop=(c == KC - 1))
                    nc.vector.tensor_add(ot[0:sz, n0:n0 + nsz],
                                         ps[0:sz, 0:nsz],
                                         pjb[0:sz, n0:n0 + nsz])
                    nc.sync.dma_start(out=out.ap()[t0:t0 + sz, n0:n0 + nsz],
                                      in_=ot[0:sz, n0:n0 + nsz])

            return step

        safe_t = (G - 1) * PW
        proj_tiles = list(range(0, T, 128))
        early = [t for t in proj_tiles if t + 128 <= safe_t]
        n_a = len(pending)
        n_p = len(early)
        assert n_a == 36  # late-tile drain points below index this layout

        def drain_until(k):
            drain(k - (n_a - len(pending)))

        for i, t0 in enumerate(early):
            proj_step(t0)()
            want = ((i + 1) * n_a) // max(n_p, 1)
            done = n_a - len(pending)
            drain(want - done)
        # late tiles as soon as their attn_T columns are transposed:
        # step 23 = av_tr(img0, it1), 34 = av_tr(img1, it0), 35 = the rest
        drain_until(24)
        proj_step(2688)()
        proj_step(2816)()
        drain_until(35)
        proj_step(2944)()
        drain(len(pending))
        proj_step(3072)()

    nc.compile()
    return nc


def _get_graph():
    global _GRAPH
    if _GRAPH is None:
        _GRAPH = _build()
    return _GRAPH


def kernel(x, qkv_w, qkv_b, proj_w, proj_b, rel_bias_table, rel_index):
    global LAST_EXEC_NS
    FP8NP = ml_dtypes.float8_e4m3
    x = np.asarray(x, dtype=np.float32)
    qkv_w = np.asarray(qkv_w, dtype=np.float32)
    qkv_b = np.asarray(qkv_b, dtype=np.float32)
    proj_w = np.asarray(proj_w, dtype=np.float32)
    proj_b = np.asarray(proj_b, dtype=np.float32)
    rel_bias_table = np.asarray(rel_bias_table, dtype=np.float32)
    rel_index = np.asarray(rel_index)

    # qkv weights: fp8 hi/lo split at scale SW, [D, 2, 3D]
    wT = np.ascontiguousarray(qkv_w.T) * SW
    wh = wT.astype(FP8NP)
    wl = (wT - wh.astype(np.float32)).astype(FP8NP)
    # [128, KC, 2, 3D]: partition-major so the per-m-col DMA merges to 3 dims
    w8 = np.stack([wh, wl], axis=1)               # [D, 2, 3D]
    w8 = w8.reshape(KC, 128, 2, 3 * D).transpose(1, 0, 2, 3)
    w8 = np.ascontiguousarray(w8)
    # per-m-group bias columns for q [128, 12]; q groups carry the score
    # scale AND the fp8 quantization scale SQ8 (qf is pre-scaled).
    # k bias is dropped on device (softmax-invariant), cols 6-11 unused.
    qkb = np.empty((128, 12), dtype=np.float32)
    for m in range(12):
        col = qkv_b[m * 128:(m + 1) * 128]
        qkb[:, m] = col * SCALE * SQ8 if m < 6 else col
    vbias = qkv_b[2 * D:3 * D]
    wprojT = np.ascontiguousarray(proj_w.T).astype(ml_dtypes.bfloat16)
    # v bias folded into the projection bias: out = (at + bv) @ WpT + bp
    pjb_eff = proj_b + vbias @ wprojT.astype(np.float32)
    # dense rel-pos bias -> [h, j(key), i(query)], exponentiated, packed into
    # the [128, 394] two-key-tile layout (rows 70:128 of cols 197:394 unused)
    bias = rel_bias_table[rel_index]  # [N, N, H]
    biasTh = np.transpose(bias, (2, 1, 0)).astype(np.float32)
    ebias = np.ones((H, 128, PW), dtype=np.float32)
    ebias[:, 0:128, 0:N] = np.exp(biasTh[:, 0:128, :])
    ebias[:, 0:69, N:PW] = np.exp(biasTh[:, 128:N, :])
    ebias = ebias.astype(ml_dtypes.bfloat16)

    nc = _get_graph()
    in_maps = []
    for i in range(NCORES):
        xs = x[i * BL:(i + 1) * BL].reshape(T, D)
        xT = np.ascontiguousarray(xs.T) * SX
        xh = xT.astype(FP8NP)
        xl = (xT - xh.astype(np.float32)).astype(FP8NP)
        # [G, 128, KC, 2, 400]: partition-major, token dim zero-padded to
        # 400 so each per-pair fetch is one contiguous 4800B run/partition
        xhl = np.stack([xh, xl], axis=1)          # [D, 2, T]
        xhl = xhl.reshape(KC, 128, 2, G, PW).transpose(3, 1, 0, 2, 4)
        xpad = np.zeros((G, 128, KC, 2, 400), dtype=FP8NP)
        xpad[:, :, :, :, 0:PW] = xhl
        in_maps.append({
            "xhl": xpad,
            "w8": w8,
            "qkb": qkb,
            "wproj": wprojT,
            "bproj": pjb_eff,
            "ebias": ebias,
        })
    res = run_bass_kernel_spmd(nc, in_maps, core_ids=list(range(NCORES)))
    LAST_EXEC_NS = res.exec_time_ns
    outs = [np.asarray(res.results[i]["out"], dtype=np.float32)
            for i in range(NCORES)]
    return np.concatenate([o.reshape(BL, N, D) for o in outs], axis=0)


# revision 34
# speedup vs baseline: 1.0452x; 1.0452x over previous
"""ViT attention block with relative position bias, SPMD over 8 TRN2 NeuronCores.

Sharding: data-parallel over batch (B=128 -> 16 images per core), weights and
bias table replicated. No collectives.

v8 design (per core, 16 images = 3152 tokens):
  - q/k GEMM in fp8 (e4m3), DoubleRow perf mode, x error-split only
    ((X_hi+X_lo)@W8, 6 passes); v GEMM token-major with the full hi/lo
    split (9 passes), landing in v_aug [tok, 12, 65] (65th col = ones for
    softmax denominators). v bias is folded into the proj bias on the host
    (attn rows sum to 1), so the v evac is a pure scale (DVE/ACT).
  - scores in fp8 DoubleRow: q evacuated as (hi, lo) fp8 pair at scale SQ8
    (near-exact), k single-quantized at SK8; k's qkv bias dropped
    (softmax-invariant per query). 1 DR pass per key tile instead of 2
    bf16 passes. exp on ACT with scale 1/(SQ8*SK8); times exp(bias) on DVE.
  - AV token-major bf16 with denominator column; reciprocal + normalize on
    DVE; PE transposes to feature-major attn_T [128, 6, T] bf16 via a
    dedicated single-bank psum pool (decoupled from the qkv/proj pool).
  - projection bf16 from attn_T; bias add on DVE.
  - all parameter loads issued up front across the SP/ACT/Pool DMA queues;
    x fetched as one contiguous 4800B/partition transfer per pair (token
    dim padded to 400 on the host), prefetched two pairs ahead (3 buffers).
  - attention of pair g-1 interleaves with the qkv m-groups of pair g,
    with AV batches spread between the two half-head score batches; the
    last pair's attention interleaves with early proj tiles, and the late
    proj tiles start as soon as their attn_T columns are transposed.
"""

import sys

import numpy as np

sys.path.insert(0, "/opt/trn_rl_repo")

import ml_dtypes  # noqa: E402

import concourse.bass as bass  # noqa: E402
import concourse.mybir as mybir  # noqa: E402
import concourse.tile as tile  # noqa: E402
from concourse import bacc  # noqa: E402
from concourse.bass_utils import run_bass_kernel_spmd  # noqa: E402
from concourse.masks import make_identity  # noqa: E402

NCORES = 8
B = 128
N = 197
D = 768
H = 12
HD = 64
BL = B // NCORES          # 16 images per core
T = BL * N                # 3152 tokens per core
G = BL // 2               # 8 image pairs
PW = 2 * N                # 394 tokens per pair
KC = D // 128             # 6 contraction chunks
SCALE = HD ** -0.5
SX = 16.0                 # fp8 scale for x
SW = 256.0                # fp8 scale for qkv weights
PS_SCL = 1.0 / (SX * SW)  # psum de-scale
SQ8 = 512.0               # fp8 scale for scaled-q (hi/lo split)
SK8 = 64.0                # fp8 scale for k (single quant)
E_SCL = 1.0 / (SQ8 * SK8)  # descale applied inside the score exp
XSPLIT_QK = True          # q/k GEMM: x split only, w single-quantized
F32 = mybir.dt.float32
BF16 = mybir.dt.bfloat16
FP8 = mybir.dt.float8e4
EXP = mybir.ActivationFunctionType.Exp
IDENT = mybir.ActivationFunctionType.Identity
DR = mybir.MatmulPerfMode.DoubleRow
MUL = mybir.AluOpType.mult
ADD = mybir.AluOpType.add
SUB = mybir.AluOpType.subtract

LAST_EXEC_NS = None
_GRAPH = None


def _bcast_ap(ap_1d, parts):
    """[n] DRAM AP -> [parts, n] AP replicated across partitions."""
    return bass.AP(tensor=ap_1d.tensor, offset=ap_1d.offset,
                   ap=[[0, parts]] + [list(d) for d in ap_1d.ap])


def _free_bcast(ap3, count):
    """[p, h, 1] AP -> [p, h, count] AP with step-0 last dim."""
    dims = [list(d) for d in ap3.ap]
    dims[-1] = [0, count]
    return bass.AP(tensor=ap3.tensor, offset=ap3.offset, ap=dims)


def _dup2(ap2):
    """[p, m] AP -> [p, 2, m] AP with stride-0 k-tile dim (DoubleRow dup)."""
    dims = [list(d) for d in ap2.ap]
    return bass.AP(tensor=ap2.tensor, offset=ap2.offset,
                   ap=[dims[0], [0, 2], dims[1]])


def _build():
    nc = bacc.Bacc("TRN2", target_bir_lowering=False, debug=False,
                   num_devices=NCORES)
    xhl = nc.declare_dram_parameter("xhl", [G, 128, KC, 2, 400], FP8,
                                    isOutput=False)
    w8 = nc.declare_dram_parameter("w8", [128, KC, 2, 3 * D], FP8,
                                   isOutput=False)
    qkb = nc.declare_dram_parameter("qkb", [128, 12], F32, isOutput=False)
    wproj = nc.declare_dram_parameter("wproj", [D, D], BF16, isOutput=False)
    bproj = nc.declare_dram_parameter("bproj", [D], F32, isOutput=False)
    ebias = nc.declare_dram_parameter("ebias", [H, 128, PW], BF16,
                                      isOutput=False)
    out = nc.declare_dram_parameter("out", [T, D], F32, isOutput=True)

    w8r = w8.ap()
    from contextlib import ExitStack
    with tile.TileContext(nc) as tc, ExitStack() as ctx:
        wpool = ctx.enter_context(tc.tile_pool(name="weights", bufs=1))
        xpool = ctx.enter_context(tc.tile_pool(name="xg", bufs=3))
        qkpool = ctx.enter_context(tc.tile_pool(name="qkg", bufs=2))
        qfpool = ctx.enter_context(tc.tile_pool(name="qf", bufs=2))
        vapool = ctx.enter_context(tc.tile_pool(name="vaug", bufs=8))
        ptpool = ctx.enter_context(tc.tile_pool(name="pt", bufs=2))
        atpool = ctx.enter_context(tc.tile_pool(name="at", bufs=4))
        rcpool = ctx.enter_context(tc.tile_pool(name="rcp", bufs=8))
        opool = ctx.enter_context(tc.tile_pool(name="osb", bufs=3))
        ps_big = ctx.enter_context(tc.tile_pool(name="psbig", bufs=3,
                                                space="PSUM"))
        ps_tr = ctx.enter_context(tc.tile_pool(name="pstr", bufs=1,
                                               space="PSUM"))  # transposes
        ps_v = ctx.enter_context(tc.tile_pool(name="psv", bufs=1,
                                              space="PSUM"))  # 1 bank
        ps_sc = ctx.enter_context(tc.tile_pool(name="pssc", bufs=2,
                                               space="PSUM"))
        ps_av = ctx.enter_context(tc.tile_pool(name="psav", bufs=1,
                                               space="PSUM"))

        # ---- persistent weights / constants ----
        # All parameter loads are issued up front, spread over the four DMA
        # queues (SP / Pool / ACT / DVE) so they run concurrently and the
        # pair-0 GEMMs are not starved behind a serial queue.
        w8t = wpool.tile([128, KC, 2, 3 * D], FP8, tag="w8", name="w8")
        qkbt = wpool.tile([128, 12], F32, tag="qkb")
        ident = wpool.tile([128, 128], BF16, tag="ident", name="ident")
        attn_T = wpool.tile([128, KC, T + 16], BF16, tag="attnT",
                            name="attnT")
        eb_all = wpool.tile([128, H, PW], BF16, tag="eball", name="eball")
        w_pj = wpool.tile([128, KC, D], BF16, tag="wproj", name="wproj")
        pjb = wpool.tile([128, D], F32, tag="pjb", name="pjb")

        # ---- qkv steps for one pair ----
        vmap = {}
        qkg_map = {}
        xg_tiles = {}

        def _fetch_xg(g, queue=None):
            # token dim padded to 400 on the host so the transfer is one
            # contiguous 4800B run per partition (and the hi/lo stride is
            # 16B-aligned for the dual-fp8 ldweights restriction)
            xg = xpool.tile([128, KC, 2, 400], FP8, tag="xg",
                            name=f"xg{g}")
            (queue or nc.sync).dma_start(out=xg[:], in_=xhl.ap()[g])
            xg_tiles[g] = xg

        # critical path first: q weights m0-m3 (SP) and x of pair 0 (ACT)
        # in parallel. ACT gets ONLY xg0 — its queue must stay clear for
        # the m-group evacs (each DMA issue holds the sequencer ~1.2us).
        # Pool carries qkb/v/wproj; k/ebias/pjb trail on SP.
        nc.sync.dma_start(out=w8t[:, :, :, 0:512], in_=w8r[:, :, :, 0:512])
        _fetch_xg(0, queue=nc.scalar)
        nc.gpsimd.dma_start(out=qkbt[:], in_=qkb.ap())
        nc.sync.dma_start(out=w8t[:, :, :, 512:D], in_=w8r[:, :, :, 512:D])
        nc.gpsimd.dma_start(out=w8t[:, :, :, 2 * D:3 * D],
                            in_=w8r[:, :, :, 2 * D:3 * D])
        _fetch_xg(1)
        nc.sync.dma_start(out=w8t[:, :, :, D:2 * D],
                          in_=w8r[:, :, :, D:2 * D])
        nc.sync.dma_start(out=eb_all[:],
                          in_=ebias.ap().rearrange("h p t -> p h t"))
        nc.sync.dma_start(out=pjb[:], in_=_bcast_ap(bproj.ap()[:], 128))
        nc.sync.dma_start(
            out=w_pj[:],
            in_=wproj.ap().rearrange("(c p) n -> p c n", p=128))
        make_identity(nc, ident[:])

        def qkv_steps(g):
            if g + 2 < G:
                _fetch_xg(g + 2)  # prefetch x two pairs ahead (3 buffers)
            xg = xg_tiles.pop(g)
            qkg = []
            qkg_map[g] = qkg

            def make_qk(m):
                def step():
                    ps = ps_big.tile([128, 512], F32, tag="big")
                    w_m = slice(m * 128, (m + 1) * 128)
                    for c in range(KC):
                        nc.tensor.matmul(ps[:, 0:PW],
                                         _dup2(w8t[:, c, 0, w_m]),
                                         xg[:, c, :, 0:PW],
                                         start=(c == 0),
                                         stop=(XSPLIT_QK and c == KC - 1),
                                         perf_mode=DR, skip_group_check=True)
                    if not XSPLIT_QK:
                        for p in range(KC // 2):
                            nc.tensor.matmul(ps[:, 0:PW],
                                             w8t[:, 2 * p:2 * p + 2, 1, w_m],
                                             xg[:, 2 * p:2 * p + 2, 0, 0:PW],
                                             start=False,
                                             stop=(p == KC // 2 - 1),
                                             perf_mode=DR,
                                             skip_group_check=True)
                    if m < 6:
                        # q: bf16 intermediate (pre-scaled by SQ8, bias in
                        # qkbt already carries SQ8) -> fp8 (hi, lo) pair.
                        # Pool cannot read PSUM and has no TensorScalar, so
                        # qf comes via DVE/ACT; hi is a convert, lo a sub.
                        scl = PS_SCL * SCALE * SQ8
                        qf = qfpool.tile([128, PW], BF16, tag="qf",
                                         name=f"qf{m}_{g}")
                        if m % 2 == 0:
                            nc.vector.tensor_scalar(qf[:], ps[:, 0:PW], scl,
                                                    qkbt[:, m:m + 1],
                                                    MUL, ADD)
                        else:
                            nc.scalar.activation(qf[:], ps[:, 0:PW], IDENT,
                                                 bias=qkbt[:, m:m + 1],
                                                 scale=scl)
                        q8 = qkpool.tile([128, 2, 400], FP8, tag=f"q8{m}",
                                         name=f"q8{m}_{g}")
                        if m % 2 == 0:
                            nc.scalar.activation(q8[:, 0, 0:PW], qf[:],
                                                 IDENT)
                            nc.gpsimd.tensor_sub(q8[:, 1, 0:PW], qf[:],
                                                 q8[:, 0, 0:PW])
                        else:
                            nc.gpsimd.tensor_copy(q8[:, 0, 0:PW], qf[:])
                            nc.vector.tensor_sub(q8[:, 1, 0:PW], qf[:],
                                                 q8[:, 0, 0:PW])
                        qkg.append(q8)
                    else:
                        # k: single fp8 quant, bias dropped (softmax-inv.)
                        k8 = qkpool.tile([128, 400], FP8, tag=f"k8{m}",
                                         name=f"k8{m}_{g}")
                        if m % 2 == 0:
                            nc.scalar.activation(k8[:, 0:PW], ps[:, 0:PW],
                                                 IDENT, scale=PS_SCL * SK8)
                        else:
                            nc.vector.tensor_scalar(k8[:, 0:PW], ps[:, 0:PW],
                                                    PS_SCL * SK8, None, MUL)
                        qkg.append(k8)

                return step

            def make_v(j, gidx):
                def step():
                    img, it = j // 2, j % 2
                    gi = 2 * g + img
                    t0 = img * N + it * 128
                    tsz = 128 if it == 0 else 70  # even M for dual-fp8 LW
                    n0, nsz = (0, 512) if gidx == 0 else (512, 256)
                    ps = ps_v.tile([128, 512], F32, tag="pv")
                    wv = slice(2 * D + n0, 2 * D + n0 + nsz)
                    for c in range(KC):
                        nc.tensor.matmul(
                            ps[0:tsz, 0:nsz],
                            xg[:, c, :, t0:t0 + tsz],
                            _dup2(w8t[:, c, 0, wv]),
                            start=(c == 0), stop=False,
                            perf_mode=DR, skip_group_check=True)
                    for p in range(KC // 2):
                        nc.tensor.matmul(
                            ps[0:tsz, 0:nsz],
                            xg[:, 2 * p:2 * p + 2, 0, t0:t0 + tsz],
                            w8t[:, 2 * p:2 * p + 2, 1, wv],
                            start=False, stop=(p == KC // 2 - 1),
                            perf_mode=DR, skip_group_check=True)
                    if gidx == 0:
                        va = vapool.tile([128, H, HD + 1], BF16,
                                         tag=f"va{it}", name=f"va{it}_{gi}")
                        nc.gpsimd.memset(va[0:tsz, :, HD:HD + 1], 1.0)
                        if it == 0:
                            vmap[gi] = [va, None]
                        else:
                            vmap[gi][1] = va
                    va = vmap[gi][it]
                    hs = slice(0, 8) if gidx == 0 else (slice(8, 12))
                    # v bias is folded into the proj bias on the host, so
                    # the evac is a pure scale; alternate DVE/ACT.
                    ps3 = ps[0:tsz, 0:nsz].rearrange("p (h d) -> p h d",
                                                     d=HD)
                    if (j + gidx) % 2 == 0:
                        nc.vector.tensor_scalar(va[0:tsz, hs, 0:HD], ps3,
                                                PS_SCL, None, MUL)
                    else:
                        nc.scalar.activation(va[0:tsz, hs, 0:HD], ps3,
                                             IDENT, scale=PS_SCL)

                return step

            steps = [make_qk(m) for m in range(12)]
            # spread the 8 v half-tiles between qk m-groups to hide evac
            # latency (after m==1 has kicked the v-weight DMA on pair 0)
            vsteps = [make_v(j, gx) for j in range(4) for gx in range(2)]
            for i, vs in enumerate(reversed(vsteps)):
                steps.insert(12 - i, vs)
            return steps

        # ---- attention steps for one pair ----
        def attn_steps(g):
            qkg = qkg_map.pop(g)
            steps = []
            pts = {}

            def score_step(img, h):
                def step():
                    if h == 0:
                        pts[img] = [
                            ptpool.tile([128, 6, PW], BF16, tag="pt0",
                                        name=f"pt0_{2 * g + img}"),
                            ptpool.tile([128, 6, PW], BF16, tag="pt1",
                                        name=f"pt1_{2 * g + img}")]
                    pt = pts[img][h // 6]
                    co = img * N
                    mq = h // 2
                    ro = (h % 2) * 64
                    mk = 6 + h // 2
                    q8 = qkg[mq]
                    k8 = qkg[mk]
                    ps = ps_sc.tile([128, PW], F32, tag="sc")
                    nc.tensor.matmul(ps[:, 0:N],
                                     _dup2(k8[ro:ro + 64, co:co + 128]),
                                     q8[ro:ro + 64, :, co:co + N],
                                     start=True, stop=True,
                                     perf_mode=DR, skip_group_check=True)
                    nc.tensor.matmul(ps[0:70, N:2 * N],
                                     _dup2(k8[ro:ro + 64, co + 128:co + 198]),
                                     q8[ro:ro + 64, :, co:co + N],
                                     start=True, stop=True,
                                     perf_mode=DR, skip_group_check=True)
                    hh = h % 6
                    nc.scalar.activation(pt[:, hh, :], ps[:], EXP,
                                         scale=E_SCL)
                    # all on DVE: a slow Pool multiply in the batch would
                    # gate the AV matmuls that consume the full pt half
                    nc.vector.tensor_mul(pt[:, hh, :], pt[:, hh, :],
                                         eb_all[:, h, :])

                return step

            def av_half(img, it, half):
                def step():
                    pt = pts[img][half]
                    va0, va1 = vmap[2 * g + img]
                    i0, isz = (0, 128) if it == 0 else (128, 69)
                    if half == 0:
                        ats[(img, it)] = atpool.tile(
                            [128, D], BF16, tag=f"at{it}",
                            name=f"at{it}_{g}_{img}")
                    at = ats[(img, it)]
                    av = ps_av.tile([128, 6 * 65], F32, tag="av")
                    for hh in range(6):
                        h = half * 6 + hh
                        nc.tensor.matmul(av[0:isz, hh * 65:(hh + 1) * 65],
                                         pt[:, hh, i0:i0 + isz],
                                         va0[:, h, :],
                                         start=True, stop=False)
                        nc.tensor.matmul(av[0:isz, hh * 65:(hh + 1) * 65],
                                         pt[0:69, hh, N + i0:N + i0 + isz],
                                         va1[0:69, h, :],
                                         start=False, stop=True)
                    av3 = av[0:isz].rearrange("p (h x) -> p h x", x=65)
                    rc = rcpool.tile([128, 6, 1], F32, tag="rc")
                    nc.vector.reciprocal(rc[0:isz], av3[:, :, 64:65])
                    nc.vector.tensor_mul(
                        at[0:isz, half * 384:(half + 1) * 384]
                        .rearrange("p (h x) -> p h x", x=HD),
                        av3[:, :, 0:HD],
                        _free_bcast(rc[0:isz], HD))

                return step

            def av_tr(img, it):
                def step():
                    at = ats[(img, it)]
                    gcol = g * PW + img * N
                    i0, isz = (0, 128) if it == 0 else (128, 69)
                    tcol = gcol + i0
                    # [128, 768] bf16 = 1536B fits a single psum bank
                    tp = ps_tr.tile([128, KC * 128], BF16, tag="tr")
                    for c in range(KC):
                        nc.tensor.transpose(tp[:, c * 128:c * 128 + isz],
                                            at[0:isz, c * 128:(c + 1) * 128],
                                            ident[0:isz, 0:isz])
                    nc.vector.tensor_copy(
                        attn_T[:, :, tcol:tcol + isz],
                        tp[:].rearrange("p (c t) -> p c t", t=128)
                        [:, :, 0:isz])

                return step

            ats = {}
            # interleave AV between the two half-head score batches so the
            # ACT exp chain (and the eb/normalize work) spreads across the
            # pair instead of clustering at its head
            for img in range(2):
                for h in range(6):
                    steps.append(score_step(img, h))
                if img == 1:
                    steps.append(av_half(0, 0, 1))
                    steps.append(av_half(0, 1, 1))
                    steps.append(av_tr(0, 0))
                    steps.append(av_tr(0, 1))
                for h in range(6, H):
                    steps.append(score_step(img, h))
                steps.append(av_half(img, 0, 0))
                steps.append(av_half(img, 1, 0))
            steps.append(av_half(1, 0, 1))
            steps.append(av_half(1, 1, 1))
            steps.append(av_tr(1, 0))
            steps.append(av_tr(1, 1))
            return steps

        # ---- main loop: interleave qkv(g) with attention(g-1) ----
        pending = []

        def drain(k):
            for _ in range(k):
                if pending:
                    pending.pop(0)()

        for g in range(G):
            qs = qkv_steps(g)
            n_q = len(qs)
            n_a = len(pending)
            for i, q in enumerate(qs):
                want = ((i + 1) * n_a) // n_q
                done = n_a - len(pending)
                drain(want - done)
                q()
            drain(len(pending))
            pending = attn_steps(g)

        # ---- output projection interleaved with last pair's attention ----
        def proj_step(t0):
            def step():
                sz = min(128, T - t0)
                ot = opool.tile([128, D], F32, tag="osb")
                for (n0, nsz) in ((0, 512), (512, 256)):
                    ps = ps_big.tile([128, 512], F32, tag="big")
                    for c in range(KC):
                        nc.tensor.matmul(ps[0:sz, 0:nsz],
                                         attn_T[:, c, t0:t0 + sz],
                                         w_pj[:, c, n0:n0 + nsz],
                                         start=(c == 0), stop=(c == KC - 1))
                    nc.vector.tensor_add(ot[0:sz, n0:n0 + nsz],
                                         ps[0:sz, 0:nsz],
                                         pjb[0:sz, n0:n0 + nsz])
                    nc.sync.dma_start(out=out.ap()[t0:t0 + sz, n0:n0 + nsz],
                                      in_=ot[0:sz, n0:n0 + nsz])

            return step

        safe_t = (G - 1) * PW
        proj_tiles = list(range(0, T, 128))
        early = [t for t in proj_tiles if t + 128 <= safe_t]
        n_a = len(pending)
        n_p = len(early)
        assert n_a == 36  # late-tile drain points below index this layout

        def drain_until(k):
            drain(k - (n_a - len(pending)))

        for i, t0 in enumerate(early):
            proj_step(t0)()
            want = ((i + 1) * n_a) // max(n_p, 1)
            done = n_a - len(pending)
            drain(want - done)
        # late tiles as soon as their attn_T columns are transposed:
        # step 23 = av_tr(img0, it1), 34 = av_tr(img1, it0), 35 = the rest
        drain_until(24)
        proj_step(2688)()
        proj_step(2816)()
        drain_until(35)
        proj_step(2944)()
        drain(len(pending))
        proj_step(3072)()

    nc.compile()
    return nc


def _get_graph():
    global _GRAPH
    if _GRAPH is None:
        _GRAPH = _build()
    return _GRAPH


def kernel(x, qkv_w, qkv_b, proj_w, proj_b, rel_bias_table, rel_index):
    global LAST_EXEC_NS
    FP8NP = ml_dtypes.float8_e4m3
    x = np.asarray(x, dtype=np.float32)
    qkv_w = np.asarray(qkv_w, dtype=np.float32)
    qkv_b = np.asarray(qkv_b, dtype=np.float32)
    proj_w = np.asarray(proj_w, dtype=np.float32)
    proj_b = np.asarray(proj_b, dtype=np.float32)
    rel_bias_table = np.asarray(rel_bias_table, dtype=np.float32)
    rel_index = np.asarray(rel_index)

    # qkv weights: fp8 hi/lo split at scale SW, [D, 2, 3D]
    wT = np.ascontiguousarray(qkv_w.T) * SW
    wh = wT.astype(FP8NP)
    wl = (wT - wh.astype(np.float32)).astype(FP8NP)
    # [128, KC, 2, 3D]: partition-major so the per-m-col DMA merges to 3 dims
    w8 = np.stack([wh, wl], axis=1)               # [D, 2, 3D]
    w8 = w8.reshape(KC, 128, 2, 3 * D).transpose(1, 0, 2, 3)
    w8 = np.ascontiguousarray(w8)
    # per-m-group bias columns for q [128, 12]; q groups carry the score
    # scale AND the fp8 quantization scale SQ8 (qf is pre-scaled).
    # k bias is dropped on device (softmax-invariant), cols 6-11 unused.
    qkb = np.empty((128, 12), dtype=np.float32)
    for m in range(12):
        col = qkv_b[m * 128:(m + 1) * 128]
        qkb[:, m] = col * SCALE * SQ8 if m < 6 else col
    vbias = qkv_b[2 * D:3 * D]
    wprojT = np.ascontiguousarray(proj_w.T).astype(ml_dtypes.bfloat16)
    # v bias folded into the projection bias: out = (at + bv) @ WpT + bp
    pjb_eff = proj_b + vbias @ wprojT.astype(np.float32)
    # dense rel-pos bias -> [h, j(key), i(query)], exponentiated, packed into
    # the [128, 394] two-key-tile layout (rows 70:128 of cols 197:394 unused)
    bias = rel_bias_table[rel_index]  # [N, N, H]
    biasTh = np.transpose(bias, (2, 1, 0)).astype(np.float32)
    ebias = np.ones((H, 128, PW), dtype=np.float32)
    ebias[:, 0:128, 0:N] = np.exp(biasTh[:, 0:128, :])
    ebias[:, 0:69, N:PW] = np.exp(biasTh[:, 128:N, :])
    ebias = ebias.astype(ml_dtypes.bfloat16)

    nc = _get_graph()
    in_maps = []
    for i in range(NCORES):
        xs = x[i * BL:(i + 1) * BL].reshape(T, D)
        xT = np.ascontiguousarray(xs.T) * SX
        xh = xT.astype(FP8NP)
        xl = (xT - xh.astype(np.float32)).astype(FP8NP)
        # [G, 128, KC, 2, 400]: partition-major, token dim zero-padded to
        # 400 so each per-pair fetch is one contiguous 4800B run/partition
        xhl = np.stack([xh, xl], axis=1)          # [D, 2, T]
        xhl = xhl.reshape(KC, 128, 2, G, PW).transpose(3, 1, 0, 2, 4)
        xpad = np.zeros((G, 128, KC, 2, 400), dtype=FP8NP)
        xpad[:, :, :, :, 0:PW] = xhl
        in_maps.append({
            "xhl": xpad,
            "w8": w8,
            "qkb": qkb,
            "wproj": wprojT,
            "bproj": pjb_eff,
            "ebias": ebias,
        })
    res = run_bass_kernel_spmd(nc, in_maps, core_ids=list(range(NCORES)))
    LAST_EXEC_NS = res.exec_time_ns
    outs = [np.asarray(res.results[i]["out"], dtype=np.float32)
            for i in range(NCORES)]
    return np.concatenate([o.reshape(BL, N, D) for o in outs], axis=0)


# revision 38
# speedup vs baseline: 1.0573x; 1.0116x over previous
"""ViT attention block with relative position bias, SPMD over 8 TRN2 NeuronCores.

Sharding: data-parallel over batch (B=128 -> 16 images per core), weights and
bias table replicated. No collectives.

v8 design (per core, 16 images = 3152 tokens):
  - q/k GEMM in fp8 (e4m3), DoubleRow perf mode, x error-split only
    ((X_hi+X_lo)@W8, 6 passes); v GEMM token-major with the full hi/lo
    split (9 passes), landing in v_aug [tok, 12, 65] (65th col = ones for
    softmax denominators). v bias is folded into the proj bias on the host
    (attn rows sum to 1), so the v evac is a pure scale (DVE/ACT).
  - scores in fp8 DoubleRow: q evacuated as (hi, lo) fp8 pair at scale SQ8
    (near-exact), k single-quantized at SK8; k's qkv bias dropped
    (softmax-invariant per query). 1 DR pass per key tile instead of 2
    bf16 passes. exp on ACT with scale 1/(SQ8*SK8); times exp(bias) on DVE.
  - AV token-major bf16 with denominator column; reciprocal + normalize on
    DVE; PE transposes to feature-major attn_T [128, 6, T] bf16 via a
    dedicated single-bank psum pool (decoupled from the qkv/proj pool).
  - projection bf16 from attn_T; bias add on DVE.
  - all parameter loads issued up front across the SP/ACT/Pool DMA queues;
    x fetched as one contiguous 4800B/partition transfer per pair (token
    dim padded to 400 on the host), prefetched two pairs ahead (3 buffers).
  - attention of pair g-1 interleaves with the qkv m-groups of pair g,
    with AV batches spread between the two half-head score batches; the
    last pair's attention interleaves with early proj tiles, and the late
    proj tiles start as soon as their attn_T columns are transposed.
"""

import sys

import numpy as np

sys.path.insert(0, "/opt/trn_rl_repo")

import ml_dtypes  # noqa: E402

import concourse.bass as bass  # noqa: E402
import concourse.mybir as mybir  # noqa: E402
import concourse.tile as tile  # noqa: E402
from concourse import bacc  # noqa: E402
from concourse.bass_utils import run_bass_kernel_spmd  # noqa: E402
from concourse.masks import make_identity  # noqa: E402

NCORES = 8
B = 128
N = 197
D = 768
H = 12
HD = 64
BL = B // NCORES          # 16 images per core
T = BL * N                # 3152 tokens per core
G = BL // 2               # 8 image pairs
PW = 2 * N                # 394 tokens per pair
KC = D // 128             # 6 contraction chunks
SCALE = HD ** -0.5
SX = 16.0                 # fp8 scale for x
SW = 256.0                # fp8 scale for qkv weights
PS_SCL = 1.0 / (SX * SW)  # psum de-scale
SQ8 = 512.0               # fp8 scale for scaled-q (hi/lo split)
SK8 = 64.0                # fp8 scale for k (single quant)
E_SCL = 1.0 / (SQ8 * SK8)  # descale applied inside the score exp
XSPLIT_QK = True          # q/k GEMM: x split only, w single-quantized
F32 = mybir.dt.float32
BF16 = mybir.dt.bfloat16
FP8 = mybir.dt.float8e4
EXP = mybir.ActivationFunctionType.Exp
IDENT = mybir.ActivationFunctionType.Identity
DR = mybir.MatmulPerfMode.DoubleRow
MUL = mybir.AluOpType.mult
ADD = mybir.AluOpType.add
SUB = mybir.AluOpType.subtract

LAST_EXEC_NS = None
_GRAPH = None


def _bcast_ap(ap_1d, parts):
    """[n] DRAM AP -> [parts, n] AP replicated across partitions."""
    return bass.AP(tensor=ap_1d.tensor, offset=ap_1d.offset,
                   ap=[[0, parts]] + [list(d) for d in ap_1d.ap])


def _free_bcast(ap3, count):
    """[p, h, 1] AP -> [p, h, count] AP with step-0 last dim."""
    dims = [list(d) for d in ap3.ap]
    dims[-1] = [0, count]
    return bass.AP(tensor=ap3.tensor, offset=ap3.offset, ap=dims)


def _dup2(ap2):
    """[p, m] AP -> [p, 2, m] AP with stride-0 k-tile dim (DoubleRow dup)."""
    dims = [list(d) for d in ap2.ap]
    return bass.AP(tensor=ap2.tensor, offset=ap2.offset,
                   ap=[dims[0], [0, 2], dims[1]])


def _build():
    nc = bacc.Bacc("TRN2", target_bir_lowering=False, debug=False,
                   num_devices=NCORES)
    xhl = nc.declare_dram_parameter("xhl", [G, 128, KC, 2, 400], FP8,
                                    isOutput=False)
    w8 = nc.declare_dram_parameter("w8", [128, KC, 2, 3 * D], FP8,
                                   isOutput=False)
    qkb = nc.declare_dram_parameter("qkb", [128, 12], F32, isOutput=False)
    wproj = nc.declare_dram_parameter("wproj", [D, D], BF16, isOutput=False)
    bproj = nc.declare_dram_parameter("bproj", [D], F32, isOutput=False)
    ebias = nc.declare_dram_parameter("ebias", [H, 128, PW], BF16,
                                      isOutput=False)
    out = nc.declare_dram_parameter("out", [T, D], F32, isOutput=True)

    w8r = w8.ap()
    from contextlib import ExitStack
    with tile.TileContext(nc) as tc, ExitStack() as ctx:
        wpool = ctx.enter_context(tc.tile_pool(name="weights", bufs=1))
        xpool = ctx.enter_context(tc.tile_pool(name="xg", bufs=3))
        qkpool = ctx.enter_context(tc.tile_pool(name="qkg", bufs=2))
        qfpool = ctx.enter_context(tc.tile_pool(name="qf", bufs=3))
        vapool = ctx.enter_context(tc.tile_pool(name="vaug", bufs=8))
        ptpool = ctx.enter_context(tc.tile_pool(name="pt", bufs=2))
        atpool = ctx.enter_context(tc.tile_pool(name="at", bufs=4))
        rcpool = ctx.enter_context(tc.tile_pool(name="rcp", bufs=8))
        opool = ctx.enter_context(tc.tile_pool(name="osb", bufs=3))
        ps_big = ctx.enter_context(tc.tile_pool(name="psbig", bufs=3,
                                                space="PSUM"))
        ps_tr = ctx.enter_context(tc.tile_pool(name="pstr", bufs=1,
                                               space="PSUM"))  # transposes
        ps_v = ctx.enter_context(tc.tile_pool(name="psv", bufs=1,
                                              space="PSUM"))  # 1 bank
        ps_sc = ctx.enter_context(tc.tile_pool(name="pssc", bufs=2,
                                               space="PSUM"))
        ps_av = ctx.enter_context(tc.tile_pool(name="psav", bufs=1,
                                               space="PSUM"))

        # ---- persistent weights / constants ----
        # All parameter loads are issued up front, spread over the four DMA
        # queues (SP / Pool / ACT / DVE) so they run concurrently and the
        # pair-0 GEMMs are not starved behind a serial queue.
        w8t = wpool.tile([128, KC, 2, 3 * D], FP8, tag="w8", name="w8")
        qkbt = wpool.tile([128, 12], F32, tag="qkb")
        ident = wpool.tile([128, 128], BF16, tag="ident", name="ident")
        attn_T = wpool.tile([128, KC, T + 16], BF16, tag="attnT",
                            name="attnT")
        eb_all = wpool.tile([128, H, PW], BF16, tag="eball", name="eball")
        w_pj = wpool.tile([128, KC, D], BF16, tag="wproj", name="wproj")
        pjb = wpool.tile([128, D], F32, tag="pjb", name="pjb")

        # ---- qkv steps for one pair ----
        vmap = {}
        qkg_map = {}
        xg_tiles = {}

        def _fetch_xg(g, queue=None):
            # token dim padded to 400 on the host so the transfer is one
            # contiguous 4800B run per partition (and the hi/lo stride is
            # 16B-aligned for the dual-fp8 ldweights restriction)
            xg = xpool.tile([128, KC, 2, 400], FP8, tag="xg",
                            name=f"xg{g}")
            (queue or nc.sync).dma_start(out=xg[:], in_=xhl.ap()[g])
            xg_tiles[g] = xg

        # critical path first: q weights m0-m3 (SP) and x of pair 0 (ACT)
        # in parallel. ACT gets ONLY xg0 — its queue must stay clear for
        # the m-group evacs (each DMA issue holds the sequencer ~1.2us).
        # Pool carries qkb/v/wproj; k/ebias/pjb trail on SP.
        nc.sync.dma_start(out=w8t[:, :, :, 0:512], in_=w8r[:, :, :, 0:512])
        _fetch_xg(0, queue=nc.scalar)
        nc.gpsimd.dma_start(out=qkbt[:], in_=qkb.ap())
        nc.sync.dma_start(out=w8t[:, :, :, 512:D], in_=w8r[:, :, :, 512:D])
        nc.gpsimd.dma_start(out=w8t[:, :, :, 2 * D:3 * D],
                            in_=w8r[:, :, :, 2 * D:3 * D])
        _fetch_xg(1)
        nc.sync.dma_start(out=w8t[:, :, :, D:2 * D],
                          in_=w8r[:, :, :, D:2 * D])
        nc.sync.dma_start(out=eb_all[:],
                          in_=ebias.ap().rearrange("h p t -> p h t"))
        nc.sync.dma_start(out=pjb[:], in_=_bcast_ap(bproj.ap()[:], 128))
        nc.sync.dma_start(
            out=w_pj[:],
            in_=wproj.ap().rearrange("(c p) n -> p c n", p=128))
        make_identity(nc, ident[:])

        def qkv_steps(g):
            if g + 2 < G:
                _fetch_xg(g + 2)  # prefetch x two pairs ahead (3 buffers)
            xg = xg_tiles.pop(g)
            qkg = []
            qkg_map[g] = qkg

            def make_qk(m):
                def step():
                    ps = ps_big.tile([128, 512], F32, tag="big")
                    w_m = slice(m * 128, (m + 1) * 128)
                    for c in range(KC):
                        nc.tensor.matmul(ps[:, 0:PW],
                                         _dup2(w8t[:, c, 0, w_m]),
                                         xg[:, c, :, 0:PW],
                                         start=(c == 0),
                                         stop=(XSPLIT_QK and c == KC - 1),
                                         perf_mode=DR, skip_group_check=True)
                    if not XSPLIT_QK:
                        for p in range(KC // 2):
                            nc.tensor.matmul(ps[:, 0:PW],
                                             w8t[:, 2 * p:2 * p + 2, 1, w_m],
                                             xg[:, 2 * p:2 * p + 2, 0, 0:PW],
                                             start=False,
                                             stop=(p == KC // 2 - 1),
                                             perf_mode=DR,
                                             skip_group_check=True)
                    if m < 6:
                        # q: bf16 intermediate (pre-scaled by SQ8, bias in
                        # qkbt already carries SQ8) -> fp8 (hi, lo) pair.
                        # Pool cannot read PSUM and has no TensorScalar, so
                        # qf comes via DVE/ACT; hi is a convert, lo a sub.
                        scl = PS_SCL * SCALE * SQ8
                        qf = qfpool.tile([128, PW], BF16, tag="qf",
                                         name=f"qf{m}_{g}")
                        if m % 2 == 0:
                            nc.vector.tensor_scalar(qf[:], ps[:, 0:PW], scl,
                                                    qkbt[:, m:m + 1],
                                                    MUL, ADD)
                        else:
                            nc.scalar.activation(qf[:], ps[:, 0:PW], IDENT,
                                                 bias=qkbt[:, m:m + 1],
                                                 scale=scl)
                        q8 = qkpool.tile([128, 2, 400], FP8, tag=f"q8{m}",
                                         name=f"q8{m}_{g}")
                        if m % 2 == 0:
                            nc.scalar.activation(q8[:, 0, 0:PW], qf[:],
                                                 IDENT)
                            nc.gpsimd.tensor_sub(q8[:, 1, 0:PW], qf[:],
                                                 q8[:, 0, 0:PW])
                        else:
                            nc.gpsimd.tensor_copy(q8[:, 0, 0:PW], qf[:])
                            nc.vector.tensor_sub(q8[:, 1, 0:PW], qf[:],
                                                 q8[:, 0, 0:PW])
                        qkg.append(q8)
                    else:
                        # k: single fp8 quant, bias dropped (softmax-inv.)
                        k8 = qkpool.tile([128, 400], FP8, tag=f"k8{m}",
                                         name=f"k8{m}_{g}")
                        if m % 2 == 0:
                            nc.scalar.activation(k8[:, 0:PW], ps[:, 0:PW],
                                                 IDENT, scale=PS_SCL * SK8)
                        else:
                            nc.vector.tensor_scalar(k8[:, 0:PW], ps[:, 0:PW],
                                                    PS_SCL * SK8, None, MUL)
                        qkg.append(k8)

                return step

            def make_v(j, gidx):
                def step():
                    img, it = j // 2, j % 2
                    gi = 2 * g + img
                    t0 = img * N + it * 128
                    tsz = 128 if it == 0 else 70  # even M for dual-fp8 LW
                    n0, nsz = (0, 512) if gidx == 0 else (512, 256)
                    ps = ps_v.tile([128, 512], F32, tag="pv")
                    wv = slice(2 * D + n0, 2 * D + n0 + nsz)
                    for c in range(KC):
                        nc.tensor.matmul(
                            ps[0:tsz, 0:nsz],
                            xg[:, c, :, t0:t0 + tsz],
                            _dup2(w8t[:, c, 0, wv]),
                            start=(c == 0), stop=False,
                            perf_mode=DR, skip_group_check=True)
                    for p in range(KC // 2):
                        nc.tensor.matmul(
                            ps[0:tsz, 0:nsz],
                            xg[:, 2 * p:2 * p + 2, 0, t0:t0 + tsz],
                            w8t[:, 2 * p:2 * p + 2, 1, wv],
                            start=False, stop=(p == KC // 2 - 1),
                            perf_mode=DR, skip_group_check=True)
                    if gidx == 0:
                        va = vapool.tile([128, H, HD + 1], BF16,
                                         tag=f"va{it}", name=f"va{it}_{gi}")
                        nc.gpsimd.memset(va[0:tsz, :, HD:HD + 1], 1.0)
                        if it == 0:
                            vmap[gi] = [va, None]
                        else:
                            vmap[gi][1] = va
                    va = vmap[gi][it]
                    hs = slice(0, 8) if gidx == 0 else (slice(8, 12))
                    # v bias is folded into the proj bias on the host, so
                    # the evac is a pure scale; alternate DVE/ACT.
                    ps3 = ps[0:tsz, 0:nsz].rearrange("p (h d) -> p h d",
                                                     d=HD)
                    if (j + gidx) % 2 == 0:
                        nc.vector.tensor_scalar(va[0:tsz, hs, 0:HD], ps3,
                                                PS_SCL, None, MUL)
                    else:
                        nc.scalar.activation(va[0:tsz, hs, 0:HD], ps3,
                                             IDENT, scale=PS_SCL)

                return step

            steps = [make_qk(m) for m in range(12)]
            # spread the 8 v half-tiles between qk m-groups to hide evac
            # latency; on pair 0 run them last — the v weights are still in
            # flight behind the q/k ones during the preload
            vsteps = [make_v(j, gx) for j in range(4) for gx in range(2)]
            if g == 0:
                steps.extend(vsteps)
            else:
                for i, vs in enumerate(reversed(vsteps)):
                    steps.insert(12 - i, vs)
            return steps

        # ---- attention steps for one pair ----
        def attn_steps(g):
            qkg = qkg_map.pop(g)
            steps = []
            pts = {}

            def score_step(img, h):
                def step():
                    if h == 0:
                        pts[img] = [
                            ptpool.tile([128, 6, PW], BF16, tag="pt0",
                                        name=f"pt0_{2 * g + img}"),
                            ptpool.tile([128, 6, PW], BF16, tag="pt1",
                                        name=f"pt1_{2 * g + img}")]
                    pt = pts[img][h // 6]
                    co = img * N
                    mq = h // 2
                    ro = (h % 2) * 64
                    mk = 6 + h // 2
                    q8 = qkg[mq]
                    k8 = qkg[mk]
                    ps = ps_sc.tile([128, PW], F32, tag="sc")
                    nc.tensor.matmul(ps[:, 0:N],
                                     _dup2(k8[ro:ro + 64, co:co + 128]),
                                     q8[ro:ro + 64, :, co:co + N],
                                     start=True, stop=True,
                                     perf_mode=DR, skip_group_check=True)
                    nc.tensor.matmul(ps[0:70, N:2 * N],
                                     _dup2(k8[ro:ro + 64, co + 128:co + 198]),
                                     q8[ro:ro + 64, :, co:co + N],
                                     start=True, stop=True,
                                     perf_mode=DR, skip_group_check=True)
                    hh = h % 6
                    nc.scalar.activation(pt[:, hh, :], ps[:], EXP,
                                         scale=E_SCL)
                    # all on DVE: a slow Pool multiply in the batch would
                    # gate the AV matmuls that consume the full pt half
                    nc.vector.tensor_mul(pt[:, hh, :], pt[:, hh, :],
                                         eb_all[:, h, :])

                return step

            def av_half(img, it, half):
                def step():
                    pt = pts[img][half]
                    va0, va1 = vmap[2 * g + img]
                    i0, isz = (0, 128) if it == 0 else (128, 69)
                    if half == 0:
                        ats[(img, it)] = atpool.tile(
                            [128, D], BF16, tag=f"at{it}",
                            name=f"at{it}_{g}_{img}")
                    at = ats[(img, it)]
                    av = ps_av.tile([128, 6 * 65], F32, tag="av")
                    for hh in range(6):
                        h = half * 6 + hh
                        nc.tensor.matmul(av[0:isz, hh * 65:(hh + 1) * 65],
                                         pt[:, hh, i0:i0 + isz],
                                         va0[:, h, :],
                                         start=True, stop=False)
                        nc.tensor.matmul(av[0:isz, hh * 65:(hh + 1) * 65],
                                         pt[0:69, hh, N + i0:N + i0 + isz],
                                         va1[0:69, h, :],
                                         start=False, stop=True)
                    av3 = av[0:isz].rearrange("p (h x) -> p h x", x=65)
                    rc = rcpool.tile([128, 6, 1], F32, tag="rc")
                    nc.vector.reciprocal(rc[0:isz], av3[:, :, 64:65])
                    nc.vector.tensor_mul(
                        at[0:isz, half * 384:(half + 1) * 384]
                        .rearrange("p (h x) -> p h x", x=HD),
                        av3[:, :, 0:HD],
                        _free_bcast(rc[0:isz], HD))

                return step

            def av_tr(img, it):
                def step():
                    at = ats[(img, it)]
                    gcol = g * PW + img * N
                    i0, isz = (0, 128) if it == 0 else (128, 69)
                    tcol = gcol + i0
                    # [128, 768] bf16 = 1536B fits a single psum bank
                    tp = ps_tr.tile([128, KC * 128], BF16, tag="tr")
                    for c in range(KC):
                        nc.tensor.transpose(tp[:, c * 128:c * 128 + isz],
                                            at[0:isz, c * 128:(c + 1) * 128],
                                            ident[0:isz, 0:isz])
                    nc.vector.tensor_copy(
                        attn_T[:, :, tcol:tcol + isz],
                        tp[:].rearrange("p (c t) -> p c t", t=128)
                        [:, :, 0:isz])

                return step

            ats = {}
            # interleave AV between the two half-head score batches so the
            # ACT exp chain (and the eb/normalize work) spreads across the
            # pair instead of clustering at its head
            for img in range(2):
                for h in range(6):
                    steps.append(score_step(img, h))
                if img == 1:
                    steps.append(av_half(0, 0, 1))
                    steps.append(av_half(0, 1, 1))
                    steps.append(av_tr(0, 0))
                    steps.append(av_tr(0, 1))
                for h in range(6, H):
                    steps.append(score_step(img, h))
                steps.append(av_half(img, 0, 0))
                steps.append(av_half(img, 1, 0))
            steps.append(av_half(1, 0, 1))
            steps.append(av_half(1, 1, 1))
            steps.append(av_tr(1, 0))
            steps.append(av_tr(1, 1))
            return steps

        # ---- main loop: interleave qkv(g) with attention(g-1) ----
        pending = []

        def drain(k):
            for _ in range(k):
                if pending:
                    pending.pop(0)()

        for g in range(G):
            qs = qkv_steps(g)
            n_q = len(qs)
            n_a = len(pending)
            for i, q in enumerate(qs):
                want = ((i + 1) * n_a) // n_q
                done = n_a - len(pending)
                drain(want - done)
                q()
            drain(len(pending))
            pending = attn_steps(g)

        # ---- output projection interleaved with last pair's attention ----
        def proj_step(t0):
            def step():
                sz = min(128, T - t0)
                ot = opool.tile([128, D], F32, tag="osb")
                for (n0, nsz) in ((0, 512), (512, 256)):
                    ps = ps_big.tile([128, 512], F32, tag="big")
                    for c in range(KC):
                        nc.tensor.matmul(ps[0:sz, 0:nsz],
                                         attn_T[:, c, t0:t0 + sz],
                                         w_pj[:, c, n0:n0 + nsz],
                                         start=(c == 0), stop=(c == KC - 1))
                    nc.vector.tensor_add(ot[0:sz, n0:n0 + nsz],
                                         ps[0:sz, 0:nsz],
                                         pjb[0:sz, n0:n0 + nsz])
                    nc.sync.dma_start(out=out.ap()[t0:t0 + sz, n0:n0 + nsz],
                                      in_=ot[0:sz, n0:n0 + nsz])

            return step

        safe_t = (G - 1) * PW
        proj_tiles = list(range(0, T, 128))
        early = [t for t in proj_tiles if t + 128 <= safe_t]
        n_a = len(pending)
        n_p = len(early)
        assert n_a == 36  # late-tile drain points below index this layout

        def drain_until(k):
            drain(k - (n_a - len(pending)))

        for i, t0 in enumerate(early):
            proj_step(t0)()
            want = ((i + 1) * n_a) // max(n_p, 1)
            done = n_a - len(pending)
            drain(want - done)
        # late tiles as soon as their attn_T columns are transposed:
        # step 23 = av_tr(img0, it1), 34 = av_tr(img1, it0), 35 = the rest
        drain_until(24)
        proj_step(2688)()
        proj_step(2816)()
        drain_until(35)
        proj_step(2944)()
        drain(len(pending))
        proj_step(3072)()

    nc.compile()
    return nc


def _get_graph():
    global _GRAPH
    if _GRAPH is None:
        _GRAPH = _build()
    return _GRAPH


def kernel(x, qkv_w, qkv_b, proj_w, proj_b, rel_bias_table, rel_index):
    global LAST_EXEC_NS
    FP8NP = ml_dtypes.float8_e4m3
    x = np.asarray(x, dtype=np.float32)
    qkv_w = np.asarray(qkv_w, dtype=np.float32)
    qkv_b = np.asarray(qkv_b, dtype=np.float32)
    proj_w = np.asarray(proj_w, dtype=np.float32)
    proj_b = np.asarray(proj_b, dtype=np.float32)
    rel_bias_table = np.asarray(rel_bias_table, dtype=np.float32)
    rel_index = np.asarray(rel_index)

    # qkv weights: fp8 hi/lo split at scale SW, [D, 2, 3D]
    wT = np.ascontiguousarray(qkv_w.T) * SW
    wh = wT.astype(FP8NP)
    wl = (wT - wh.astype(np.float32)).astype(FP8NP)
    # [128, KC, 2, 3D]: partition-major so the per-m-col DMA merges to 3 dims
    w8 = np.stack([wh, wl], axis=1)               # [D, 2, 3D]
    w8 = w8.reshape(KC, 128, 2, 3 * D).transpose(1, 0, 2, 3)
    w8 = np.ascontiguousarray(w8)
    # per-m-group bias columns for q [128, 12]; q groups carry the score
    # scale AND the fp8 quantization scale SQ8 (qf is pre-scaled).
    # k bias is dropped on device (softmax-invariant), cols 6-11 unused.
    qkb = np.empty((128, 12), dtype=np.float32)
    for m in range(12):
        col = qkv_b[m * 128:(m + 1) * 128]
        qkb[:, m] = col * SCALE * SQ8 if m < 6 else col
    vbias = qkv_b[2 * D:3 * D]
    wprojT = np.ascontiguousarray(proj_w.T).astype(ml_dtypes.bfloat16)
    # v bias folded into the projection bias: out = (at + bv) @ WpT + bp
    pjb_eff = proj_b + vbias @ wprojT.astype(np.float32)
    # dense rel-pos bias -> [h, j(key), i(query)], exponentiated, packed into
    # the [128, 394] two-key-tile layout (rows 70:128 of cols 197:394 unused)
    bias = rel_bias_table[rel_index]  # [N, N, H]
    biasTh = np.transpose(bias, (2, 1, 0)).astype(np.float32)
    ebias = np.ones((H, 128, PW), dtype=np.float32)
    ebias[:, 0:128, 0:N] = np.exp(biasTh[:, 0:128, :])
    ebias[:, 0:69, N:PW] = np.exp(biasTh[:, 128:N, :])
    ebias = ebias.astype(ml_dtypes.bfloat16)

    nc = _get_graph()
    in_maps = []
    for i in range(NCORES):
        xs = x[i * BL:(i + 1) * BL].reshape(T, D)
        xT = np.ascontiguousarray(xs.T) * SX
        xh = xT.astype(FP8NP)
        xl = (xT - xh.astype(np.float32)).astype(FP8NP)
        # [G, 128, KC, 2, 400]: partition-major, token dim zero-padded to
        # 400 so each per-pair fetch is one contiguous 4800B run/partition
        xhl = np.stack([xh, xl], axis=1)          # [D, 2, T]
        xhl = xhl.reshape(KC, 128, 2, G, PW).transpose(3, 1, 0, 2, 4)
        xpad = np.zeros((G, 128, KC, 2, 400), dtype=FP8NP)
        xpad[:, :, :, :, 0:PW] = xhl
        in_maps.append({
            "xhl": xpad,
            "w8": w8,
            "qkb": qkb,
            "wproj": wprojT,
            "bproj": pjb_eff,
            "ebias": ebias,
        })
    res = run_bass_kernel_spmd(nc, in_maps, core_ids=list(range(NCORES)))
    LAST_EXEC_NS = res.exec_time_ns
    outs = [np.asarray(res.results[i]["out"], dtype=np.float32)
            for i in range(NCORES)]
    return np.concatenate([o.reshape(BL, N, D) for o in outs], axis=0)


# revision 44
# speedup vs baseline: 1.0667x; 1.0088x over previous
"""ViT attention block with relative position bias, SPMD over 8 TRN2 NeuronCores.

Sharding: data-parallel over batch (B=128 -> 16 images per core), weights and
bias table replicated. No collectives.

v8 design (per core, 16 images = 3152 tokens):
  - q/k GEMM in fp8 (e4m3), DoubleRow perf mode, x error-split only
    ((X_hi+X_lo)@W8, 6 passes); v GEMM token-major with the full hi/lo
    split (9 passes), landing in v_aug [tok, 12, 65] (65th col = ones for
    softmax denominators). v bias is folded into the proj bias on the host
    (attn rows sum to 1), so the v evac is a pure scale (DVE/ACT).
  - scores in fp8 DoubleRow: q evacuated as (hi, lo) fp8 pair at scale SQ8
    (near-exact), k single-quantized at SK8; k's qkv bias dropped
    (softmax-invariant per query). 1 DR pass per key tile instead of 2
    bf16 passes. exp on ACT with scale 1/(SQ8*SK8); times exp(bias) on DVE.
  - AV token-major bf16 with denominator column; reciprocal + normalize on
    DVE; PE transposes to feature-major attn_T [128, 6, T] bf16 via a
    dedicated single-bank psum pool (decoupled from the qkv/proj pool).
  - projection bf16 from attn_T; bias add on DVE.
  - all parameter loads issued up front across the SP/ACT/Pool DMA queues;
    x fetched as one contiguous 4800B/partition transfer per pair (token
    dim padded to 400 on the host), prefetched two pairs ahead (3 buffers).
  - attention of pair g-1 interleaves with the qkv m-groups of pair g,
    with AV batches spread between the two half-head score batches; the
    last pair's attention interleaves with early proj tiles, and the late
    proj tiles start as soon as their attn_T columns are transposed.
"""

import sys

import numpy as np

sys.path.insert(0, "/opt/trn_rl_repo")

import ml_dtypes  # noqa: E402

import concourse.bass as bass  # noqa: E402
import concourse.mybir as mybir  # noqa: E402
import concourse.tile as tile  # noqa: E402
from concourse import bacc  # noqa: E402
from concourse.bass_utils import run_bass_kernel_spmd  # noqa: E402
from concourse.masks import make_identity  # noqa: E402

NCORES = 8
B = 128
N = 197
D = 768
H = 12
HD = 64
BL = B // NCORES          # 16 images per core
T = BL * N                # 3152 tokens per core
G = BL // 2               # 8 image pairs
PW = 2 * N                # 394 tokens per pair
KC = D // 128             # 6 contraction chunks
SCALE = HD ** -0.5
SX = 16.0                 # fp8 scale for x
SW = 256.0                # fp8 scale for qkv weights
PS_SCL = 1.0 / (SX * SW)  # psum de-scale
SQ8 = 512.0               # fp8 scale for scaled-q (hi/lo split)
SK8 = 64.0                # fp8 scale for k (single quant)
E_SCL = 1.0 / (SQ8 * SK8)  # descale applied inside the score exp
XSPLIT_QK = True          # q/k GEMM: x split only, w single-quantized
F32 = mybir.dt.float32
BF16 = mybir.dt.bfloat16
FP8 = mybir.dt.float8e4
EXP = mybir.ActivationFunctionType.Exp
IDENT = mybir.ActivationFunctionType.Identity
DR = mybir.MatmulPerfMode.DoubleRow
MUL = mybir.AluOpType.mult
ADD = mybir.AluOpType.add
SUB = mybir.AluOpType.subtract

LAST_EXEC_NS = None
_GRAPH = None


def _bcast_ap(ap_1d, parts):
    """[n] DRAM AP -> [parts, n] AP replicated across partitions."""
    return bass.AP(tensor=ap_1d.tensor, offset=ap_1d.offset,
                   ap=[[0, parts]] + [list(d) for d in ap_1d.ap])


def _free_bcast(ap3, count):
    """[p, h, 1] AP -> [p, h, count] AP with step-0 last dim."""
    dims = [list(d) for d in ap3.ap]
    dims[-1] = [0, count]
    return bass.AP(tensor=ap3.tensor, offset=ap3.offset, ap=dims)


def _dup2(ap2):
    """[p, m] AP -> [p, 2, m] AP with stride-0 k-tile dim (DoubleRow dup)."""
    dims = [list(d) for d in ap2.ap]
    return bass.AP(tensor=ap2.tensor, offset=ap2.offset,
                   ap=[dims[0], [0, 2], dims[1]])


def _build():
    nc = bacc.Bacc("TRN2", target_bir_lowering=False, debug=False,
                   num_devices=NCORES)
    xhl = nc.declare_dram_parameter("xhl", [G, 128, KC, 2, 400], FP8,
                                    isOutput=False)
    w8 = nc.declare_dram_parameter("w8", [128, KC, 2, 3 * D], FP8,
                                   isOutput=False)
    qkb = nc.declare_dram_parameter("qkb", [128, 12], F32, isOutput=False)
    wproj = nc.declare_dram_parameter("wproj", [D, D], BF16, isOutput=False)
    bproj = nc.declare_dram_parameter("bproj", [D], F32, isOutput=False)
    ebias = nc.declare_dram_parameter("ebias", [H, 128, PW], BF16,
                                      isOutput=False)
    out = nc.declare_dram_parameter("out", [T, D], F32, isOutput=True)

    w8r = w8.ap()
    from contextlib import ExitStack
    with tile.TileContext(nc) as tc, ExitStack() as ctx:
        wpool = ctx.enter_context(tc.tile_pool(name="weights", bufs=1))
        xpool = ctx.enter_context(tc.tile_pool(name="xg", bufs=3))
        qkpool = ctx.enter_context(tc.tile_pool(name="qkg", bufs=2))
        qfpool = ctx.enter_context(tc.tile_pool(name="qf", bufs=3))
        vapool = ctx.enter_context(tc.tile_pool(name="vaug", bufs=8))
        ptpool = ctx.enter_context(tc.tile_pool(name="pt", bufs=2))
        atpool = ctx.enter_context(tc.tile_pool(name="at", bufs=4))
        rcpool = ctx.enter_context(tc.tile_pool(name="rcp", bufs=8))
        opool = ctx.enter_context(tc.tile_pool(name="osb", bufs=3))
        ps_big = ctx.enter_context(tc.tile_pool(name="psbig", bufs=3,
                                                space="PSUM"))
        ps_tr = ctx.enter_context(tc.tile_pool(name="pstr", bufs=1,
                                               space="PSUM"))  # transposes
        ps_v = ctx.enter_context(tc.tile_pool(name="psv", bufs=1,
                                              space="PSUM"))  # 1 bank
        ps_sc = ctx.enter_context(tc.tile_pool(name="pssc", bufs=2,
                                               space="PSUM"))
        ps_av = ctx.enter_context(tc.tile_pool(name="psav", bufs=1,
                                               space="PSUM"))

        # ---- persistent weights / constants ----
        # All parameter loads are issued up front, spread over the four DMA
        # queues (SP / Pool / ACT / DVE) so they run concurrently and the
        # pair-0 GEMMs are not starved behind a serial queue.
        w8t = wpool.tile([128, KC, 2, 3 * D], FP8, tag="w8", name="w8")
        qkbt = wpool.tile([128, 12], F32, tag="qkb")
        ident = wpool.tile([128, 128], BF16, tag="ident", name="ident")
        attn_T = wpool.tile([128, KC, T + 16], BF16, tag="attnT",
                            name="attnT")
        eb_all = wpool.tile([128, H, PW], BF16, tag="eball", name="eball")
        w_pj = wpool.tile([128, KC, D], BF16, tag="wproj", name="wproj")
        pjb = wpool.tile([128, D], F32, tag="pjb", name="pjb")

        # ---- qkv steps for one pair ----
        vmap = {}
        qkg_map = {}
        xg_tiles = {}

        def _fetch_xg(g, queue=None):
            # token dim padded to 400 on the host so the transfer is one
            # contiguous 4800B run per partition (and the hi/lo stride is
            # 16B-aligned for the dual-fp8 ldweights restriction)
            xg = xpool.tile([128, KC, 2, 400], FP8, tag="xg",
                            name=f"xg{g}")
            (queue or nc.sync).dma_start(out=xg[:], in_=xhl.ap()[g])
            xg_tiles[g] = xg

        # critical path first: q weights m0-m3 (SP) and x of pair 0 (ACT)
        # in parallel. ACT gets ONLY xg0 — its queue must stay clear for
        # the m-group evacs (each DMA issue holds the sequencer ~1.2us).
        # Pool carries qkb/v/wproj; k/ebias/pjb trail on SP.
        nc.sync.dma_start(out=w8t[:, :, :, 0:512], in_=w8r[:, :, :, 0:512])
        _fetch_xg(0, queue=nc.scalar)
        nc.gpsimd.dma_start(out=qkbt[:], in_=qkb.ap())
        nc.sync.dma_start(out=w8t[:, :, :, 512:D], in_=w8r[:, :, :, 512:D])
        nc.gpsimd.dma_start(out=w8t[:, :, :, 2 * D:3 * D],
                            in_=w8r[:, :, :, 2 * D:3 * D])
        _fetch_xg(1)
        nc.sync.dma_start(out=w8t[:, :, :, D:2 * D],
                          in_=w8r[:, :, :, D:2 * D])
        nc.sync.dma_start(out=eb_all[:],
                          in_=ebias.ap().rearrange("h p t -> p h t"))
        nc.sync.dma_start(out=pjb[:], in_=_bcast_ap(bproj.ap()[:], 128))
        nc.sync.dma_start(
            out=w_pj[:],
            in_=wproj.ap().rearrange("(c p) n -> p c n", p=128))
        make_identity(nc, ident[:])

        def qkv_steps(g):
            if g + 2 < G:
                _fetch_xg(g + 2)  # prefetch x two pairs ahead (3 buffers)
            xg = xg_tiles.pop(g)
            qkg = []
            qkg_map[g] = qkg

            def make_qk(m):
                def step():
                    ps = ps_big.tile([128, 512], F32, tag="big")
                    w_m = slice(m * 128, (m + 1) * 128)
                    for c in range(KC):
                        nc.tensor.matmul(ps[:, 0:PW],
                                         _dup2(w8t[:, c, 0, w_m]),
                                         xg[:, c, :, 0:PW],
                                         start=(c == 0),
                                         stop=(XSPLIT_QK and c == KC - 1),
                                         perf_mode=DR, skip_group_check=True)
                    if not XSPLIT_QK:
                        for p in range(KC // 2):
                            nc.tensor.matmul(ps[:, 0:PW],
                                             w8t[:, 2 * p:2 * p + 2, 1, w_m],
                                             xg[:, 2 * p:2 * p + 2, 0, 0:PW],
                                             start=False,
                                             stop=(p == KC // 2 - 1),
                                             perf_mode=DR,
                                             skip_group_check=True)
                    if m < 6:
                        # q: bf16 intermediate (pre-scaled by SQ8, bias in
                        # qkbt already carries SQ8) -> fp8 (hi, lo) pair.
                        # Pool cannot read PSUM and has no TensorScalar, so
                        # qf comes via DVE/ACT; hi is a convert, lo a sub.
                        scl = PS_SCL * SCALE * SQ8
                        qf = qfpool.tile([128, PW], BF16, tag="qf",
                                         name=f"qf{m}_{g}")
                        if m % 2 == 0:
                            nc.vector.tensor_scalar(qf[:], ps[:, 0:PW], scl,
                                                    qkbt[:, m:m + 1],
                                                    MUL, ADD)
                        else:
                            nc.scalar.activation(qf[:], ps[:, 0:PW], IDENT,
                                                 bias=qkbt[:, m:m + 1],
                                                 scale=scl)
                        q8 = qkpool.tile([128, 2, 400], FP8, tag=f"q8{m}",
                                         name=f"q8{m}_{g}")
                        # hi on Pool for both parities: ACT is the scarce
                        # engine in steady state (the score exp chain)
                        nc.gpsimd.tensor_copy(q8[:, 0, 0:PW], qf[:])
                        if m % 2 == 0:
                            nc.gpsimd.tensor_sub(q8[:, 1, 0:PW], qf[:],
                                                 q8[:, 0, 0:PW])
                        else:
                            nc.vector.tensor_sub(q8[:, 1, 0:PW], qf[:],
                                                 q8[:, 0, 0:PW])
                        qkg.append(q8)
                    else:
                        # k: single fp8 quant, bias dropped (softmax-inv.)
                        k8 = qkpool.tile([128, 400], FP8, tag=f"k8{m}",
                                         name=f"k8{m}_{g}")
                        if m % 2 == 0:
                            nc.scalar.activation(k8[:, 0:PW], ps[:, 0:PW],
                                                 IDENT, scale=PS_SCL * SK8)
                        else:
                            nc.vector.tensor_scalar(k8[:, 0:PW], ps[:, 0:PW],
                                                    PS_SCL * SK8, None, MUL)
                        qkg.append(k8)

                return step

            def make_v(j, gidx):
                def step():
                    img, it = j // 2, j % 2
                    gi = 2 * g + img
                    t0 = img * N + it * 128
                    tsz = 128 if it == 0 else 70  # even M for dual-fp8 LW
                    n0, nsz = (0, 512) if gidx == 0 else (512, 256)
                    ps = ps_v.tile([128, 512], F32, tag="pv")
                    wv = slice(2 * D + n0, 2 * D + n0 + nsz)
                    for c in range(KC):
                        nc.tensor.matmul(
                            ps[0:tsz, 0:nsz],
                            xg[:, c, :, t0:t0 + tsz],
                            _dup2(w8t[:, c, 0, wv]),
                            start=(c == 0), stop=False,
                            perf_mode=DR, skip_group_check=True)
                    for p in range(KC // 2):
                        nc.tensor.matmul(
                            ps[0:tsz, 0:nsz],
                            xg[:, 2 * p:2 * p + 2, 0, t0:t0 + tsz],
                            w8t[:, 2 * p:2 * p + 2, 1, wv],
                            start=False, stop=(p == KC // 2 - 1),
                            perf_mode=DR, skip_group_check=True)
                    if gidx == 0:
                        va = vapool.tile([128, H, HD + 1], BF16,
                                         tag=f"va{it}", name=f"va{it}_{gi}")
                        nc.gpsimd.memset(va[0:tsz, :, HD:HD + 1], 1.0)
                        if it == 0:
                            vmap[gi] = [va, None]
                        else:
                            vmap[gi][1] = va
                    va = vmap[gi][it]
                    hs = slice(0, 8) if gidx == 0 else (slice(8, 12))
                    # v bias is folded into the proj bias on the host, so
                    # the evac is a pure scale; alternate DVE/ACT.
                    ps3 = ps[0:tsz, 0:nsz].rearrange("p (h d) -> p h d",
                                                     d=HD)
                    if (j + gidx) % 2 == 0:
                        nc.vector.tensor_scalar(va[0:tsz, hs, 0:HD], ps3,
                                                PS_SCL, None, MUL)
                    else:
                        nc.scalar.activation(va[0:tsz, hs, 0:HD], ps3,
                                             IDENT, scale=PS_SCL)

                return step

            steps = [make_qk(m) for m in range(12)]
            # spread the 8 v half-tiles between qk m-groups to hide evac
            # latency; on pair 0 run them last — the v weights are still in
            # flight behind the q/k ones during the preload
            vsteps = [make_v(j, gx) for j in range(4) for gx in range(2)]
            if g == 0:
                steps.extend(vsteps)
            else:
                for i, vs in enumerate(reversed(vsteps)):
                    steps.insert(12 - i, vs)
            return steps

        # ---- attention steps for one pair ----
        def attn_steps(g):
            qkg = qkg_map.pop(g)
            steps = []
            pts = {}

            def score_step(img, h):
                def step():
                    if h == 0:
                        pts[img] = [
                            ptpool.tile([128, 6, PW], BF16, tag="pt0",
                                        name=f"pt0_{2 * g + img}"),
                            ptpool.tile([128, 6, PW], BF16, tag="pt1",
                                        name=f"pt1_{2 * g + img}")]
                    pt = pts[img][h // 6]
                    co = img * N
                    mq = h // 2
                    ro = (h % 2) * 64
                    mk = 6 + h // 2
                    q8 = qkg[mq]
                    k8 = qkg[mk]
                    ps = ps_sc.tile([128, PW], F32, tag="sc")
                    nc.tensor.matmul(ps[:, 0:N],
                                     _dup2(k8[ro:ro + 64, co:co + 128]),
                                     q8[ro:ro + 64, :, co:co + N],
                                     start=True, stop=True,
                                     perf_mode=DR, skip_group_check=True)
                    nc.tensor.matmul(ps[0:70, N:2 * N],
                                     _dup2(k8[ro:ro + 64, co + 128:co + 198]),
                                     q8[ro:ro + 64, :, co:co + N],
                                     start=True, stop=True,
                                     perf_mode=DR, skip_group_check=True)
                    hh = h % 6
                    nc.scalar.activation(pt[:, hh, :], ps[:], EXP,
                                         scale=E_SCL)
                    # Pool takes the FIRST head of each half (its ~877ns
                    # finishes before DVE's remaining five, so it never
                    # gates the AV that consumes the full pt half)
                    if hh == 0:
                        nc.gpsimd.tensor_mul(pt[:, hh, :], pt[:, hh, :],
                                             eb_all[:, h, :])
                    else:
                        nc.vector.tensor_mul(pt[:, hh, :], pt[:, hh, :],
                                             eb_all[:, h, :])

                return step

            def av_half(img, it, half):
                def step():
                    pt = pts[img][half]
                    va0, va1 = vmap[2 * g + img]
                    i0, isz = (0, 128) if it == 0 else (128, 69)
                    if half == 0:
                        ats[(img, it)] = atpool.tile(
                            [128, D], BF16, tag=f"at{it}",
                            name=f"at{it}_{g}_{img}")
                    at = ats[(img, it)]
                    av = ps_av.tile([128, 6 * 65], F32, tag="av")
                    for hh in range(6):
                        h = half * 6 + hh
                        nc.tensor.matmul(av[0:isz, hh * 65:(hh + 1) * 65],
                                         pt[:, hh, i0:i0 + isz],
                                         va0[:, h, :],
                                         start=True, stop=False)
                        nc.tensor.matmul(av[0:isz, hh * 65:(hh + 1) * 65],
                                         pt[0:69, hh, N + i0:N + i0 + isz],
                                         va1[0:69, h, :],
                                         start=False, stop=True)
                    av3 = av[0:isz].rearrange("p (h x) -> p h x", x=65)
                    rc = rcpool.tile([128, 6, 1], F32, tag="rc")
                    nc.vector.reciprocal(rc[0:isz], av3[:, :, 64:65])
                    nc.vector.tensor_mul(
                        at[0:isz, half * 384:(half + 1) * 384]
                        .rearrange("p (h x) -> p h x", x=HD),
                        av3[:, :, 0:HD],
                        _free_bcast(rc[0:isz], HD))

                return step

            def av_tr(img, it):
                def step():
                    at = ats[(img, it)]
                    gcol = g * PW + img * N
                    i0, isz = (0, 128) if it == 0 else (128, 69)
                    tcol = gcol + i0
                    # [128, 768] bf16 = 1536B fits a single psum bank
                    tp = ps_tr.tile([128, KC * 128], BF16, tag="tr")
                    for c in range(KC):
                        nc.tensor.transpose(tp[:, c * 128:c * 128 + isz],
                                            at[0:isz, c * 128:(c + 1) * 128],
                                            ident[0:isz, 0:isz])
                    nc.vector.tensor_copy(
                        attn_T[:, :, tcol:tcol + isz],
                        tp[:].rearrange("p (c t) -> p c t", t=128)
                        [:, :, 0:isz])

                return step

            ats = {}
            # interleave AV between the two half-head score batches so the
            # ACT exp chain (and the eb/normalize work) spreads across the
            # pair instead of clustering at its head
            for img in range(2):
                for h in range(6):
                    steps.append(score_step(img, h))
                if img == 1:
                    steps.append(av_half(0, 0, 1))
                    steps.append(av_half(0, 1, 1))
                    steps.append(av_tr(0, 0))
                    steps.append(av_tr(0, 1))
                for h in range(6, H):
                    steps.append(score_step(img, h))
                steps.append(av_half(img, 0, 0))
                steps.append(av_half(img, 1, 0))
            steps.append(av_half(1, 0, 1))
            steps.append(av_half(1, 1, 1))
            steps.append(av_tr(1, 0))
            steps.append(av_tr(1, 1))
            return steps

        # ---- main loop: interleave qkv(g) with attention(g-1) ----
        pending = []

        def drain(k):
            for _ in range(k):
                if pending:
                    pending.pop(0)()

        for g in range(G):
            qs = qkv_steps(g)
            n_q = len(qs)
            n_a = len(pending)
            for i, q in enumerate(qs):
                want = ((i + 1) * n_a) // n_q
                done = n_a - len(pending)
                drain(want - done)
                q()
            drain(len(pending))
            pending = attn_steps(g)

        # ---- output projection interleaved with last pair's attention ----
        def proj_step(t0):
            def step():
                sz = min(128, T - t0)
                ot = opool.tile([128, D], F32, tag="osb")
                for (n0, nsz) in ((0, 512), (512, 256)):
                    ps = ps_big.tile([128, 512], F32, tag="big")
                    for c in range(KC):
                        nc.tensor.matmul(ps[0:sz, 0:nsz],
                                         attn_T[:, c, t0:t0 + sz],
                                         w_pj[:, c, n0:n0 + nsz],
                                         start=(c == 0), stop=(c == KC - 1))
                    nc.vector.tensor_add(ot[0:sz, n0:n0 + nsz],
                                         ps[0:sz, 0:nsz],
                                         pjb[0:sz, n0:n0 + nsz])
                    nc.sync.dma_start(out=out.ap()[t0:t0 + sz, n0:n0 + nsz],
                                      in_=ot[0:sz, n0:n0 + nsz])

            return step

        safe_t = (G - 1) * PW
        proj_tiles = list(range(0, T, 128))
        early = [t for t in proj_tiles if t + 128 <= safe_t]
        n_a = len(pending)
        n_p = len(early)
        assert n_a == 36  # late-tile drain points below index this layout

        def drain_until(k):
            drain(k - (n_a - len(pending)))

        for i, t0 in enumerate(early):
            proj_step(t0)()
            want = ((i + 1) * n_a) // max(n_p, 1)
            done = n_a - len(pending)
            drain(want - done)
        # late tiles as soon as their attn_T columns are transposed:
        # step 23 = av_tr(img0, it1), 34 = av_tr(img1, it0), 35 = the rest
        drain_until(24)
        proj_step(2688)()
        proj_step(2816)()
        drain_until(35)
        proj_step(2944)()
        drain(len(pending))
        proj_step(3072)()

    nc.compile()
    return nc


def _get_graph():
    global _GRAPH
    if _GRAPH is None:
        _GRAPH = _build()
    return _GRAPH


def kernel(x, qkv_w, qkv_b, proj_w, proj_b, rel_bias_table, rel_index):
    global LAST_EXEC_NS
    FP8NP = ml_dtypes.float8_e4m3
    x = np.asarray(x, dtype=np.float32)
    qkv_w = np.asarray(qkv_w, dtype=np.float32)
    qkv_b = np.asarray(qkv_b, dtype=np.float32)
    proj_w = np.asarray(proj_w, dtype=np.float32)
    proj_b = np.asarray(proj_b, dtype=np.float32)
    rel_bias_table = np.asarray(rel_bias_table, dtype=np.float32)
    rel_index = np.asarray(rel_index)

    # qkv weights: fp8 hi/lo split at scale SW, [D, 2, 3D]
    wT = np.ascontiguousarray(qkv_w.T) * SW
    wh = wT.astype(FP8NP)
    wl = (wT - wh.astype(np.float32)).astype(FP8NP)
    # [128, KC, 2, 3D]: partition-major so the per-m-col DMA merges to 3 dims
    w8 = np.stack([wh, wl], axis=1)               # [D, 2, 3D]
    w8 = w8.reshape(KC, 128, 2, 3 * D).transpose(1, 0, 2, 3)
    w8 = np.ascontiguousarray(w8)
    # per-m-group bias columns for q [128, 12]; q groups carry the score
    # scale AND the fp8 quantization scale SQ8 (qf is pre-scaled).
    # k bias is dropped on device (softmax-invariant), cols 6-11 unused.
    qkb = np.empty((128, 12), dtype=np.float32)
    for m in range(12):
        col = qkv_b[m * 128:(m + 1) * 128]
        qkb[:, m] = col * SCALE * SQ8 if m < 6 else col
    vbias = qkv_b[2 * D:3 * D]
    wprojT = np.ascontiguousarray(proj_w.T).astype(ml_dtypes.bfloat16)
    # v bias folded into the projection bias: out = (at + bv) @ WpT + bp
    pjb_eff = proj_b + vbias @ wprojT.astype(np.float32)
    # dense rel-pos bias -> [h, j(key), i(query)], exponentiated, packed into
    # the [128, 394] two-key-tile layout (rows 70:128 of cols 197:394 unused)
    bias = rel_bias_table[rel_index]  # [N, N, H]
    biasTh = np.transpose(bias, (2, 1, 0)).astype(np.float32)
    ebias = np.ones((H, 128, PW), dtype=np.float32)
    ebias[:, 0:128, 0:N] = np.exp(biasTh[:, 0:128, :])
    ebias[:, 0:69, N:PW] = np.exp(biasTh[:, 128:N, :])
    ebias = ebias.astype(ml_dtypes.bfloat16)

    nc = _get_graph()
    in_maps = []
    for i in range(NCORES):
        xs = x[i * BL:(i + 1) * BL].reshape(T, D)
        xT = np.ascontiguousarray(xs.T) * SX
        xh = xT.astype(FP8NP)
        xl = (xT - xh.astype(np.float32)).astype(FP8NP)
        # [G, 128, KC, 2, 400]: partition-major, token dim zero-padded to
        # 400 so each per-pair fetch is one contiguous 4800B run/partition
        xhl = np.stack([xh, xl], axis=1)          # [D, 2, T]
        xhl = xhl.reshape(KC, 128, 2, G, PW).transpose(3, 1, 0, 2, 4)
        xpad = np.zeros((G, 128, KC, 2, 400), dtype=FP8NP)
        xpad[:, :, :, :, 0:PW] = xhl
        in_maps.append({
            "xhl": xpad,
            "w8": w8,
            "qkb": qkb,
            "wproj": wprojT,
            "bproj": pjb_eff,
            "ebias": ebias,
        })
    res = run_bass_kernel_spmd(nc, in_maps, core_ids=list(range(NCORES)))
    LAST_EXEC_NS = res.exec_time_ns
    outs = [np.asarray(res.results[i]["out"], dtype=np.float32)
            for i in range(NCORES)]
    return np.concatenate([o.reshape(BL, N, D) for o in outs], axis=0)


# revision 47
# speedup vs baseline: 1.0738x; 1.0067x over previous
"""ViT attention block with relative position bias, SPMD over 8 TRN2 NeuronCores.

Sharding: data-parallel over batch (B=128 -> 16 images per core), weights and
bias table replicated. No collectives.

v8 design (per core, 16 images = 3152 tokens):
  - q/k GEMM in fp8 (e4m3), DoubleRow perf mode, x error-split only
    ((X_hi+X_lo)@W8, 6 passes); v GEMM token-major with the full hi/lo
    split (9 passes), landing in v_aug [tok, 12, 65] (65th col = ones for
    softmax denominators). v bias is folded into the proj bias on the host
    (attn rows sum to 1), so the v evac is a pure scale (DVE/ACT).
  - scores in fp8 DoubleRow: q evacuated as (hi, lo) fp8 pair at scale SQ8
    (near-exact), k single-quantized at SK8; k's qkv bias dropped
    (softmax-invariant per query). 1 DR pass per key tile instead of 2
    bf16 passes. exp on ACT with scale 1/(SQ8*SK8); times exp(bias) on DVE.
  - AV token-major bf16 with denominator column; reciprocal + normalize on
    DVE; PE transposes to feature-major attn_T [128, 6, T] bf16 via a
    dedicated single-bank psum pool (decoupled from the qkv/proj pool).
  - projection bf16 from attn_T; bias add on DVE.
  - all parameter loads issued up front across the SP/ACT/Pool DMA queues;
    x fetched as one contiguous 4800B/partition transfer per pair (token
    dim padded to 400 on the host), prefetched two pairs ahead (3 buffers).
  - attention of pair g-1 interleaves with the qkv m-groups of pair g,
    with AV batches spread between the two half-head score batches; the
    last pair's attention interleaves with early proj tiles, and the late
    proj tiles start as soon as their attn_T columns are transposed.
"""

import sys

import numpy as np

sys.path.insert(0, "/opt/trn_rl_repo")

import ml_dtypes  # noqa: E402

import concourse.bass as bass  # noqa: E402
import concourse.mybir as mybir  # noqa: E402
import concourse.tile as tile  # noqa: E402
from concourse import bacc  # noqa: E402
from concourse.bass_utils import run_bass_kernel_spmd  # noqa: E402
from concourse.masks import make_identity  # noqa: E402

NCORES = 8
B = 128
N = 197
D = 768
H = 12
HD = 64
BL = B // NCORES          # 16 images per core
T = BL * N                # 3152 tokens per core
G = BL // 2               # 8 image pairs
PW = 2 * N                # 394 tokens per pair
KC = D // 128             # 6 contraction chunks
SCALE = HD ** -0.5
SX = 16.0                 # fp8 scale for x
SW = 256.0                # fp8 scale for qkv weights
PS_SCL = 1.0 / (SX * SW)  # psum de-scale
SQ8 = 512.0               # fp8 scale for scaled-q (hi/lo split)
SK8 = 64.0                # fp8 scale for k (single quant)
E_SCL = 1.0 / (SQ8 * SK8)  # descale applied inside the score exp
XSPLIT_QK = True          # q/k GEMM: x split only, w single-quantized
F32 = mybir.dt.float32
BF16 = mybir.dt.bfloat16
FP8 = mybir.dt.float8e4
EXP = mybir.ActivationFunctionType.Exp
IDENT = mybir.ActivationFunctionType.Identity
DR = mybir.MatmulPerfMode.DoubleRow
MUL = mybir.AluOpType.mult
ADD = mybir.AluOpType.add
SUB = mybir.AluOpType.subtract

LAST_EXEC_NS = None
_GRAPH = None


def _bcast_ap(ap_1d, parts):
    """[n] DRAM AP -> [parts, n] AP replicated across partitions."""
    return bass.AP(tensor=ap_1d.tensor, offset=ap_1d.offset,
                   ap=[[0, parts]] + [list(d) for d in ap_1d.ap])


def _free_bcast(ap3, count):
    """[p, h, 1] AP -> [p, h, count] AP with step-0 last dim."""
    dims = [list(d) for d in ap3.ap]
    dims[-1] = [0, count]
    return bass.AP(tensor=ap3.tensor, offset=ap3.offset, ap=dims)


def _dup2(ap2):
    """[p, m] AP -> [p, 2, m] AP with stride-0 k-tile dim (DoubleRow dup)."""
    dims = [list(d) for d in ap2.ap]
    return bass.AP(tensor=ap2.tensor, offset=ap2.offset,
                   ap=[dims[0], [0, 2], dims[1]])


def _build():
    nc = bacc.Bacc("TRN2", target_bir_lowering=False, debug=False,
                   num_devices=NCORES)
    xhl = nc.declare_dram_parameter("xhl", [G, 128, KC, 2, 400], FP8,
                                    isOutput=False)
    w8 = nc.declare_dram_parameter("w8", [128, KC, 2, 3 * D], FP8,
                                   isOutput=False)
    qkb = nc.declare_dram_parameter("qkb", [128, 12], F32, isOutput=False)
    wproj = nc.declare_dram_parameter("wproj", [D, D], BF16, isOutput=False)
    bproj = nc.declare_dram_parameter("bproj", [D], F32, isOutput=False)
    ebias = nc.declare_dram_parameter("ebias", [H, 128, PW], BF16,
                                      isOutput=False)
    out = nc.declare_dram_parameter("out", [T, D], F32, isOutput=True)

    w8r = w8.ap()
    from contextlib import ExitStack
    with tile.TileContext(nc) as tc, ExitStack() as ctx:
        wpool = ctx.enter_context(tc.tile_pool(name="weights", bufs=1))
        xpool = ctx.enter_context(tc.tile_pool(name="xg", bufs=3))
        qkpool = ctx.enter_context(tc.tile_pool(name="qkg", bufs=2))
        qfpool = ctx.enter_context(tc.tile_pool(name="qf", bufs=3))
        vapool = ctx.enter_context(tc.tile_pool(name="vaug", bufs=8))
        ptpool = ctx.enter_context(tc.tile_pool(name="pt", bufs=2))
        atpool = ctx.enter_context(tc.tile_pool(name="at", bufs=4))
        rcpool = ctx.enter_context(tc.tile_pool(name="rcp", bufs=8))
        opool = ctx.enter_context(tc.tile_pool(name="osb", bufs=3))
        ps_big = ctx.enter_context(tc.tile_pool(name="psbig", bufs=3,
                                                space="PSUM"))
        ps_tr = ctx.enter_context(tc.tile_pool(name="pstr", bufs=1,
                                               space="PSUM"))  # transposes
        ps_v = ctx.enter_context(tc.tile_pool(name="psv", bufs=1,
                                              space="PSUM"))  # 1 bank
        ps_sc = ctx.enter_context(tc.tile_pool(name="pssc", bufs=2,
                                               space="PSUM"))
        ps_av = ctx.enter_context(tc.tile_pool(name="psav", bufs=1,
                                               space="PSUM"))

        # ---- persistent weights / constants ----
        # All parameter loads are issued up front, spread over the four DMA
        # queues (SP / Pool / ACT / DVE) so they run concurrently and the
        # pair-0 GEMMs are not starved behind a serial queue.
        w8t = wpool.tile([128, KC, 2, 3 * D], FP8, tag="w8", name="w8")
        qkbt = wpool.tile([128, 12], F32, tag="qkb")
        ident = wpool.tile([128, 128], BF16, tag="ident", name="ident")
        attn_T = wpool.tile([128, KC, T + 16], BF16, tag="attnT",
                            name="attnT")
        eb_all = wpool.tile([128, H, PW], BF16, tag="eball", name="eball")
        w_pj = wpool.tile([128, KC, D], BF16, tag="wproj", name="wproj")
        pjb = wpool.tile([128, D], F32, tag="pjb", name="pjb")

        # ---- qkv steps for one pair ----
        vmap = {}
        qkg_map = {}
        xg_tiles = {}

        def _fetch_xg(g, queue=None):
            # token dim padded to 400 on the host so the transfer is one
            # contiguous 4800B run per partition (and the hi/lo stride is
            # 16B-aligned for the dual-fp8 ldweights restriction)
            xg = xpool.tile([128, KC, 2, 400], FP8, tag="xg",
                            name=f"xg{g}")
            (queue or nc.sync).dma_start(out=xg[:], in_=xhl.ap()[g])
            xg_tiles[g] = xg

        # critical path first: q weights m0-m3 (SP) and x of pair 0 (ACT)
        # in parallel. ACT gets ONLY xg0 — its queue must stay clear for
        # the m-group evacs (each DMA issue holds the sequencer ~1.2us).
        # Pool carries qkb/v/wproj; k/ebias/pjb trail on SP.
        nc.sync.dma_start(out=w8t[:, :, :, 0:512], in_=w8r[:, :, :, 0:512])
        _fetch_xg(0, queue=nc.scalar)
        nc.gpsimd.dma_start(out=qkbt[:], in_=qkb.ap())
        nc.sync.dma_start(out=w8t[:, :, :, 512:D], in_=w8r[:, :, :, 512:D])
        nc.gpsimd.dma_start(out=w8t[:, :, :, 2 * D:3 * D],
                            in_=w8r[:, :, :, 2 * D:3 * D])
        _fetch_xg(1)
        nc.sync.dma_start(out=w8t[:, :, :, D:2 * D],
                          in_=w8r[:, :, :, D:2 * D])
        nc.sync.dma_start(out=eb_all[:],
                          in_=ebias.ap().rearrange("h p t -> p h t"))
        nc.sync.dma_start(out=pjb[:], in_=_bcast_ap(bproj.ap()[:], 128))
        nc.sync.dma_start(
            out=w_pj[:],
            in_=wproj.ap().rearrange("(c p) n -> p c n", p=128))
        make_identity(nc, ident[:])

        def qkv_steps(g):
            if g + 2 < G:
                _fetch_xg(g + 2)  # prefetch x two pairs ahead (3 buffers)
            xg = xg_tiles.pop(g)
            qkg = []
            qkg_map[g] = qkg

            def make_qk(m):
                def step():
                    ps = ps_big.tile([128, 512], F32, tag="big")
                    w_m = slice(m * 128, (m + 1) * 128)
                    for c in range(KC):
                        nc.tensor.matmul(ps[:, 0:PW],
                                         _dup2(w8t[:, c, 0, w_m]),
                                         xg[:, c, :, 0:PW],
                                         start=(c == 0),
                                         stop=(XSPLIT_QK and c == KC - 1),
                                         perf_mode=DR, skip_group_check=True)
                    if not XSPLIT_QK:
                        for p in range(KC // 2):
                            nc.tensor.matmul(ps[:, 0:PW],
                                             w8t[:, 2 * p:2 * p + 2, 1, w_m],
                                             xg[:, 2 * p:2 * p + 2, 0, 0:PW],
                                             start=False,
                                             stop=(p == KC // 2 - 1),
                                             perf_mode=DR,
                                             skip_group_check=True)
                    if m < 6:
                        # q: bf16 intermediate (pre-scaled by SQ8, bias in
                        # qkbt already carries SQ8) -> fp8 (hi, lo) pair.
                        # Pool cannot read PSUM and has no TensorScalar, so
                        # qf comes via DVE/ACT; hi is a convert, lo a sub.
                        scl = PS_SCL * SCALE * SQ8
                        qf = qfpool.tile([128, PW], BF16, tag="qf",
                                         name=f"qf{m}_{g}")
                        # all qf on DVE and all hi/lo on Pool: ACT is the
                        # scarce engine in steady state (the score exp chain)
                        nc.vector.tensor_scalar(qf[:], ps[:, 0:PW], scl,
                                                qkbt[:, m:m + 1],
                                                MUL, ADD)
                        q8 = qkpool.tile([128, 2, 400], FP8, tag=f"q8{m}",
                                         name=f"q8{m}_{g}")
                        nc.gpsimd.tensor_copy(q8[:, 0, 0:PW], qf[:])
                        nc.gpsimd.tensor_sub(q8[:, 1, 0:PW], qf[:],
                                             q8[:, 0, 0:PW])
                        qkg.append(q8)
                    else:
                        # k: single fp8 quant, bias dropped (softmax-inv.)
                        k8 = qkpool.tile([128, 400], FP8, tag=f"k8{m}",
                                         name=f"k8{m}_{g}")
                        if m % 2 == 0:
                            nc.scalar.activation(k8[:, 0:PW], ps[:, 0:PW],
                                                 IDENT, scale=PS_SCL * SK8)
                        else:
                            nc.vector.tensor_scalar(k8[:, 0:PW], ps[:, 0:PW],
                                                    PS_SCL * SK8, None, MUL)
                        qkg.append(k8)

                return step

            def make_v(j, gidx):
                def step():
                    img, it = j // 2, j % 2
                    gi = 2 * g + img
                    t0 = img * N + it * 128
                    tsz = 128 if it == 0 else 70  # even M for dual-fp8 LW
                    n0, nsz = (0, 512) if gidx == 0 else (512, 256)
                    ps = ps_v.tile([128, 512], F32, tag="pv")
                    wv = slice(2 * D + n0, 2 * D + n0 + nsz)
                    for c in range(KC):
                        nc.tensor.matmul(
                            ps[0:tsz, 0:nsz],
                            xg[:, c, :, t0:t0 + tsz],
                            _dup2(w8t[:, c, 0, wv]),
                            start=(c == 0), stop=False,
                            perf_mode=DR, skip_group_check=True)
                    for p in range(KC // 2):
                        nc.tensor.matmul(
                            ps[0:tsz, 0:nsz],
                            xg[:, 2 * p:2 * p + 2, 0, t0:t0 + tsz],
                            w8t[:, 2 * p:2 * p + 2, 1, wv],
                            start=False, stop=(p == KC // 2 - 1),
                            perf_mode=DR, skip_group_check=True)
                    if gidx == 0:
                        va = vapool.tile([128, H, HD + 1], BF16,
                                         tag=f"va{it}", name=f"va{it}_{gi}")
                        nc.gpsimd.memset(va[0:tsz, :, HD:HD + 1], 1.0)
                        if it == 0:
                            vmap[gi] = [va, None]
                        else:
                            vmap[gi][1] = va
                    va = vmap[gi][it]
                    hs = slice(0, 8) if gidx == 0 else (slice(8, 12))
                    # v bias is folded into the proj bias on the host, so
                    # the evac is a pure scale; alternate DVE/ACT.
                    ps3 = ps[0:tsz, 0:nsz].rearrange("p (h d) -> p h d",
                                                     d=HD)
                    if (j + gidx) % 2 == 0:
                        nc.vector.tensor_scalar(va[0:tsz, hs, 0:HD], ps3,
                                                PS_SCL, None, MUL)
                    else:
                        nc.scalar.activation(va[0:tsz, hs, 0:HD], ps3,
                                             IDENT, scale=PS_SCL)

                return step

            steps = [make_qk(m) for m in range(12)]
            # spread the 8 v half-tiles between qk m-groups to hide evac
            # latency; on pair 0 run them last — the v weights are still in
            # flight behind the q/k ones during the preload
            vsteps = [make_v(j, gx) for j in range(4) for gx in range(2)]
            if g == 0:
                steps.extend(vsteps)
            else:
                for i, vs in enumerate(reversed(vsteps)):
                    steps.insert(12 - i, vs)
            return steps

        # ---- attention steps for one pair ----
        def attn_steps(g):
            qkg = qkg_map.pop(g)
            steps = []
            pts = {}

            def score_step(img, h):
                def step():
                    if h == 0:
                        pts[img] = [
                            ptpool.tile([128, 6, PW], BF16, tag="pt0",
                                        name=f"pt0_{2 * g + img}"),
                            ptpool.tile([128, 6, PW], BF16, tag="pt1",
                                        name=f"pt1_{2 * g + img}")]
                    pt = pts[img][h // 6]
                    co = img * N
                    mq = h // 2
                    ro = (h % 2) * 64
                    mk = 6 + h // 2
                    q8 = qkg[mq]
                    k8 = qkg[mk]
                    ps = ps_sc.tile([128, PW], F32, tag="sc")
                    nc.tensor.matmul(ps[:, 0:N],
                                     _dup2(k8[ro:ro + 64, co:co + 128]),
                                     q8[ro:ro + 64, :, co:co + N],
                                     start=True, stop=True,
                                     perf_mode=DR, skip_group_check=True)
                    nc.tensor.matmul(ps[0:70, N:2 * N],
                                     _dup2(k8[ro:ro + 64, co + 128:co + 198]),
                                     q8[ro:ro + 64, :, co:co + N],
                                     start=True, stop=True,
                                     perf_mode=DR, skip_group_check=True)
                    hh = h % 6
                    nc.scalar.activation(pt[:, hh, :], ps[:], EXP,
                                         scale=E_SCL)
                    # Pool takes the FIRST head of each half (its ~877ns
                    # finishes before DVE's remaining five, so it never
                    # gates the AV that consumes the full pt half)
                    if hh == 0:
                        nc.gpsimd.tensor_mul(pt[:, hh, :], pt[:, hh, :],
                                             eb_all[:, h, :])
                    else:
                        nc.vector.tensor_mul(pt[:, hh, :], pt[:, hh, :],
                                             eb_all[:, h, :])

                return step

            def av_half(img, it, half):
                def step():
                    pt = pts[img][half]
                    va0, va1 = vmap[2 * g + img]
                    i0, isz = (0, 128) if it == 0 else (128, 69)
                    if half == 0:
                        ats[(img, it)] = atpool.tile(
                            [128, D], BF16, tag=f"at{it}",
                            name=f"at{it}_{g}_{img}")
                    at = ats[(img, it)]
                    av = ps_av.tile([128, 6 * 65], F32, tag="av")
                    for hh in range(6):
                        h = half * 6 + hh
                        nc.tensor.matmul(av[0:isz, hh * 65:(hh + 1) * 65],
                                         pt[:, hh, i0:i0 + isz],
                                         va0[:, h, :],
                                         start=True, stop=False)
                        nc.tensor.matmul(av[0:isz, hh * 65:(hh + 1) * 65],
                                         pt[0:69, hh, N + i0:N + i0 + isz],
                                         va1[0:69, h, :],
                                         start=False, stop=True)
                    av3 = av[0:isz].rearrange("p (h x) -> p h x", x=65)
                    rc = rcpool.tile([128, 6, 1], F32, tag="rc")
                    nc.vector.reciprocal(rc[0:isz], av3[:, :, 64:65])
                    nc.vector.tensor_mul(
                        at[0:isz, half * 384:(half + 1) * 384]
                        .rearrange("p (h x) -> p h x", x=HD),
                        av3[:, :, 0:HD],
                        _free_bcast(rc[0:isz], HD))

                return step

            def av_tr(img, it):
                def step():
                    at = ats[(img, it)]
                    gcol = g * PW + img * N
                    i0, isz = (0, 128) if it == 0 else (128, 69)
                    tcol = gcol + i0
                    # [128, 768] bf16 = 1536B fits a single psum bank
                    tp = ps_tr.tile([128, KC * 128], BF16, tag="tr")
                    for c in range(KC):
                        nc.tensor.transpose(tp[:, c * 128:c * 128 + isz],
                                            at[0:isz, c * 128:(c + 1) * 128],
                                            ident[0:isz, 0:isz])
                    nc.vector.tensor_copy(
                        attn_T[:, :, tcol:tcol + isz],
                        tp[:].rearrange("p (c t) -> p c t", t=128)
                        [:, :, 0:isz])

                return step

            ats = {}
            # interleave AV between the two half-head score batches so the
            # ACT exp chain (and the eb/normalize work) spreads across the
            # pair instead of clustering at its head
            for img in range(2):
                for h in range(6):
                    steps.append(score_step(img, h))
                if img == 1:
                    steps.append(av_half(0, 0, 1))
                    steps.append(av_half(0, 1, 1))
                    steps.append(av_tr(0, 0))
                    steps.append(av_tr(0, 1))
                for h in range(6, H):
                    steps.append(score_step(img, h))
                steps.append(av_half(img, 0, 0))
                steps.append(av_half(img, 1, 0))
            steps.append(av_half(1, 0, 1))
            steps.append(av_half(1, 1, 1))
            steps.append(av_tr(1, 0))
            steps.append(av_tr(1, 1))
            return steps

        # ---- main loop: interleave qkv(g) with attention(g-1) ----
        pending = []

        def drain(k):
            for _ in range(k):
                if pending:
                    pending.pop(0)()

        for g in range(G):
            qs = qkv_steps(g)
            n_q = len(qs)
            n_a = len(pending)
            for i, q in enumerate(qs):
                want = ((i + 1) * n_a) // n_q
                done = n_a - len(pending)
                drain(want - done)
                q()
            drain(len(pending))
            pending = attn_steps(g)

        # ---- output projection interleaved with last pair's attention ----
        def proj_step(t0):
            def step():
                sz = min(128, T - t0)
                ot = opool.tile([128, D], F32, tag="osb")
                for (n0, nsz) in ((0, 512), (512, 256)):
                    ps = ps_big.tile([128, 512], F32, tag="big")
                    for c in range(KC):
                        nc.tensor.matmul(ps[0:sz, 0:nsz],
                                         attn_T[:, c, t0:t0 + sz],
                                         w_pj[:, c, n0:n0 + nsz],
                                         start=(c == 0), stop=(c == KC - 1))
                    nc.vector.tensor_add(ot[0:sz, n0:n0 + nsz],
                                         ps[0:sz, 0:nsz],
                                         pjb[0:sz, n0:n0 + nsz])
                    nc.sync.dma_start(out=out.ap()[t0:t0 + sz, n0:n0 + nsz],
                                      in_=ot[0:sz, n0:n0 + nsz])

            return step

        safe_t = (G - 1) * PW
        proj_tiles = list(range(0, T, 128))
        early = [t for t in proj_tiles if t + 128 <= safe_t]
        n_a = len(pending)
        n_p = len(early)
        assert n_a == 36  # late-tile drain points below index this layout

        def drain_until(k):
            drain(k - (n_a - len(pending)))

        for i, t0 in enumerate(early):
            proj_step(t0)()
            want = ((i + 1) * n_a) // max(n_p, 1)
            done = n_a - len(pending)
            drain(want - done)
        # late tiles as soon as their attn_T columns are transposed:
        # step 23 = av_tr(img0, it1), 34 = av_tr(img1, it0), 35 = the rest
        drain_until(24)
        proj_step(2688)()
        proj_step(2816)()
        drain_until(35)
        proj_step(2944)()
        drain(len(pending))
        proj_step(3072)()

    nc.compile()
    return nc


def _get_graph():
    global _GRAPH
    if _GRAPH is None:
        _GRAPH = _build()
    return _GRAPH


def kernel(x, qkv_w, qkv_b, proj_w, proj_b, rel_bias_table, rel_index):
    global LAST_EXEC_NS
    FP8NP = ml_dtypes.float8_e4m3
    x = np.asarray(x, dtype=np.float32)
    qkv_w = np.asarray(qkv_w, dtype=np.float32)
    qkv_b = np.asarray(qkv_b, dtype=np.float32)
    proj_w = np.asarray(proj_w, dtype=np.float32)
    proj_b = np.asarray(proj_b, dtype=np.float32)
    rel_bias_table = np.asarray(rel_bias_table, dtype=np.float32)
    rel_index = np.asarray(rel_index)

    # qkv weights: fp8 hi/lo split at scale SW, [D, 2, 3D]
    wT = np.ascontiguousarray(qkv_w.T) * SW
    wh = wT.astype(FP8NP)
    wl = (wT - wh.astype(np.float32)).astype(FP8NP)
    # [128, KC, 2, 3D]: partition-major so the per-m-col DMA merges to 3 dims
    w8 = np.stack([wh, wl], axis=1)               # [D, 2, 3D]
    w8 = w8.reshape(KC, 128, 2, 3 * D).transpose(1, 0, 2, 3)
    w8 = np.ascontiguousarray(w8)
    # per-m-group bias columns for q [128, 12]; q groups carry the score
    # scale AND the fp8 quantization scale SQ8 (qf is pre-scaled).
    # k bias is dropped on device (softmax-invariant), cols 6-11 unused.
    qkb = np.empty((128, 12), dtype=np.float32)
    for m in range(12):
        col = qkv_b[m * 128:(m + 1) * 128]
        qkb[:, m] = col * SCALE * SQ8 if m < 6 else col
    vbias = qkv_b[2 * D:3 * D]
    wprojT = np.ascontiguousarray(proj_w.T).astype(ml_dtypes.bfloat16)
    # v bias folded into the projection bias: out = (at + bv) @ WpT + bp
    pjb_eff = proj_b + vbias @ wprojT.astype(np.float32)
    # dense rel-pos bias -> [h, j(key), i(query)], exponentiated, packed into
    # the [128, 394] two-key-tile layout (rows 70:128 of cols 197:394 unused)
    bias = rel_bias_table[rel_index]  # [N, N, H]
    biasTh = np.transpose(bias, (2, 1, 0)).astype(np.float32)
    ebias = np.ones((H, 128, PW), dtype=np.float32)
    ebias[:, 0:128, 0:N] = np.exp(biasTh[:, 0:128, :])
    ebias[:, 0:69, N:PW] = np.exp(biasTh[:, 128:N, :])
    ebias = ebias.astype(ml_dtypes.bfloat16)

    nc = _get_graph()
    in_maps = []
    for i in range(NCORES):
        xs = x[i * BL:(i + 1) * BL].reshape(T, D)
        xT = np.ascontiguousarray(xs.T) * SX
        xh = xT.astype(FP8NP)
        xl = (xT - xh.astype(np.float32)).astype(FP8NP)
        # [G, 128, KC, 2, 400]: partition-major, token dim zero-padded to
        # 400 so each per-pair fetch is one contiguous 4800B run/partition
        xhl = np.stack([xh, xl], axis=1)          # [D, 2, T]
        xhl = xhl.reshape(KC, 128, 2, G, PW).transpose(3, 1, 0, 2, 4)
        xpad = np.zeros((G, 128, KC, 2, 400), dtype=FP8NP)
        xpad[:, :, :, :, 0:PW] = xhl
        in_maps.append({
            "xhl": xpad,
            "w8": w8,
            "qkb": qkb,
            "wproj": wprojT,
            "bproj": pjb_eff,
            "ebias": ebias,
        })
    res = run_bass_kernel_spmd(nc, in_maps, core_ids=list(range(NCORES)))
    LAST_EXEC_NS = res.exec_time_ns
    outs = [np.asarray(res.results[i]["out"], dtype=np.float32)
            for i in range(NCORES)]
    return np.concatenate([o.reshape(BL, N, D) for o in outs], axis=0)


# revision 51
# speedup vs baseline: 1.1057x; 1.0297x over previous
"""ViT attention block with relative position bias, SPMD over 8 TRN2 NeuronCores.

Sharding: data-parallel over batch (B=128 -> 16 images per core), weights and
bias table replicated. No collectives.

v8 design (per core, 16 images = 3152 tokens):
  - q/k GEMM in fp8 (e4m3), DoubleRow perf mode, x error-split only
    ((X_hi+X_lo)@W8, 6 passes); v GEMM token-major with the full hi/lo
    split (9 passes), landing in v_aug [tok, 12, 65] (65th col = ones for
    softmax denominators). v bias is folded into the proj bias on the host
    (attn rows sum to 1), so the v evac is a pure scale (DVE/ACT).
  - scores in fp8 DoubleRow: q evacuated as (hi, lo) fp8 pair at scale SQ8
    (near-exact), k single-quantized at SK8; k's qkv bias dropped
    (softmax-invariant per query). 1 DR pass per key tile instead of 2
    bf16 passes. exp on ACT with scale 1/(SQ8*SK8); times exp(bias) on DVE.
  - AV token-major bf16 with denominator column; reciprocal + normalize on
    DVE; PE transposes to feature-major attn_T [128, 6, T] bf16 via a
    dedicated single-bank psum pool (decoupled from the qkv/proj pool).
  - projection bf16 from attn_T; bias add on DVE.
  - all parameter loads issued up front across the SP/ACT/Pool DMA queues;
    x fetched as one contiguous 4800B/partition transfer per pair (token
    dim padded to 400 on the host), prefetched two pairs ahead (3 buffers).
  - attention of pair g-1 interleaves with the qkv m-groups of pair g,
    with AV batches spread between the two half-head score batches; the
    last pair's attention interleaves with early proj tiles, and the late
    proj tiles start as soon as their attn_T columns are transposed.
"""

import sys

import numpy as np

sys.path.insert(0, "/opt/trn_rl_repo")

import ml_dtypes  # noqa: E402

import concourse.bass as bass  # noqa: E402
import concourse.mybir as mybir  # noqa: E402
import concourse.tile as tile  # noqa: E402
from concourse import bacc  # noqa: E402
from concourse.bass_utils import run_bass_kernel_spmd  # noqa: E402
from concourse.masks import make_identity  # noqa: E402

NCORES = 8
B = 128
N = 197
D = 768
H = 12
HD = 64
BL = B // NCORES          # 16 images per core
T = BL * N                # 3152 tokens per core
G = BL // 2               # 8 image pairs
PW = 2 * N                # 394 tokens per pair
KC = D // 128             # 6 contraction chunks
SCALE = HD ** -0.5
SX = 16.0                 # fp8 scale for x
SW = 256.0                # fp8 scale for qkv weights
PS_SCL = 1.0 / (SX * SW)  # psum de-scale
SQ8 = 512.0               # fp8 scale for scaled-q (hi/lo split)
SK8 = 64.0                # fp8 scale for k (single quant)
E_SCL = 1.0 / (SQ8 * SK8)  # descale applied inside the score exp
XSPLIT_QK = True          # q/k GEMM: x split only, w single-quantized
F32 = mybir.dt.float32
BF16 = mybir.dt.bfloat16
FP8 = mybir.dt.float8e4
EXP = mybir.ActivationFunctionType.Exp
IDENT = mybir.ActivationFunctionType.Identity
DR = mybir.MatmulPerfMode.DoubleRow
MUL = mybir.AluOpType.mult
ADD = mybir.AluOpType.add
SUB = mybir.AluOpType.subtract

LAST_EXEC_NS = None
_GRAPH = None


def _bcast_ap(ap_1d, parts):
    """[n] DRAM AP -> [parts, n] AP replicated across partitions."""
    return bass.AP(tensor=ap_1d.tensor, offset=ap_1d.offset,
                   ap=[[0, parts]] + [list(d) for d in ap_1d.ap])


def _free_bcast(ap3, count):
    """[p, h, 1] AP -> [p, h, count] AP with step-0 last dim."""
    dims = [list(d) for d in ap3.ap]
    dims[-1] = [0, count]
    return bass.AP(tensor=ap3.tensor, offset=ap3.offset, ap=dims)


def _dup2(ap2):
    """[p, m] AP -> [p, 2, m] AP with stride-0 k-tile dim (DoubleRow dup)."""
    dims = [list(d) for d in ap2.ap]
    return bass.AP(tensor=ap2.tensor, offset=ap2.offset,
                   ap=[dims[0], [0, 2], dims[1]])


def _build():
    nc = bacc.Bacc("TRN2", target_bir_lowering=False, debug=False,
                   num_devices=NCORES)
    xhl = nc.declare_dram_parameter("xhl", [G, 128, KC, 2, 400], FP8,
                                    isOutput=False)
    w8 = nc.declare_dram_parameter("w8", [128, KC, 2, 3 * D], FP8,
                                   isOutput=False)
    qkb = nc.declare_dram_parameter("qkb", [128, 12], F32, isOutput=False)
    wproj = nc.declare_dram_parameter("wproj", [D, D], BF16, isOutput=False)
    bproj = nc.declare_dram_parameter("bproj", [D], F32, isOutput=False)
    ebias = nc.declare_dram_parameter("ebias", [H, 128, PW], BF16,
                                      isOutput=False)
    out = nc.declare_dram_parameter("out", [T, D], F32, isOutput=True)

    w8r = w8.ap()
    from contextlib import ExitStack
    with tile.TileContext(nc) as tc, ExitStack() as ctx:
        wpool = ctx.enter_context(tc.tile_pool(name="weights", bufs=1))
        xpool = ctx.enter_context(tc.tile_pool(name="xg", bufs=3))
        qkpool = ctx.enter_context(tc.tile_pool(name="qkg", bufs=2))
        qfpool = ctx.enter_context(tc.tile_pool(name="qf", bufs=3))
        vapool = ctx.enter_context(tc.tile_pool(name="vaug", bufs=8))
        ptpool = ctx.enter_context(tc.tile_pool(name="pt", bufs=2))
        atpool = ctx.enter_context(tc.tile_pool(name="at", bufs=4))
        rcpool = ctx.enter_context(tc.tile_pool(name="rcp", bufs=8))
        opool = ctx.enter_context(tc.tile_pool(name="osb", bufs=3))
        ps_big = ctx.enter_context(tc.tile_pool(name="psbig", bufs=3,
                                                space="PSUM"))
        ps_tr = ctx.enter_context(tc.tile_pool(name="pstr", bufs=1,
                                               space="PSUM"))  # transposes
        ps_v = ctx.enter_context(tc.tile_pool(name="psv", bufs=1,
                                              space="PSUM"))  # 1 bank
        ps_sc = ctx.enter_context(tc.tile_pool(name="pssc", bufs=2,
                                               space="PSUM"))
        ps_av = ctx.enter_context(tc.tile_pool(name="psav", bufs=1,
                                               space="PSUM"))

        # ---- persistent weights / constants ----
        # All parameter loads are issued up front, spread over the four DMA
        # queues (SP / Pool / ACT / DVE) so they run concurrently and the
        # pair-0 GEMMs are not starved behind a serial queue.
        w8t = wpool.tile([128, KC, 2, 3 * D], FP8, tag="w8", name="w8")
        qkbt = wpool.tile([128, 12], F32, tag="qkb")
        ident = wpool.tile([128, 128], BF16, tag="ident", name="ident")
        attn_T = wpool.tile([128, KC, T + 16], BF16, tag="attnT",
                            name="attnT")
        eb_all = wpool.tile([128, H, PW], BF16, tag="eball", name="eball")
        w_pj = wpool.tile([128, KC, D], BF16, tag="wproj", name="wproj")
        pjb = wpool.tile([128, D], F32, tag="pjb", name="pjb")

        # ---- qkv steps for one pair ----
        vmap = {}
        qkg_map = {}
        xg_tiles = {}

        def _fetch_xg(g, queue=None):
            # token dim padded to 400 on the host so the transfer is one
            # contiguous 4800B run per partition (and the hi/lo stride is
            # 16B-aligned for the dual-fp8 ldweights restriction)
            xg = xpool.tile([128, KC, 2, 400], FP8, tag="xg",
                            name=f"xg{g}")
            (queue or nc.sync).dma_start(out=xg[:], in_=xhl.ap()[g])
            xg_tiles[g] = xg

        # critical path first: q weights m0-m3 (SP) and x of pair 0 (ACT)
        # in parallel. ACT gets ONLY xg0 — its queue must stay clear for
        # the m-group evacs (each DMA issue holds the sequencer ~1.2us).
        # Pool carries qkb/v/wproj; k/ebias/pjb trail on SP.
        nc.sync.dma_start(out=w8t[:, :, :, 0:512], in_=w8r[:, :, :, 0:512])
        _fetch_xg(0, queue=nc.scalar)
        nc.gpsimd.dma_start(out=qkbt[:], in_=qkb.ap())
        nc.sync.dma_start(out=w8t[:, :, :, 512:D], in_=w8r[:, :, :, 512:D])
        nc.gpsimd.dma_start(out=w8t[:, :, :, 2 * D:3 * D],
                            in_=w8r[:, :, :, 2 * D:3 * D])
        _fetch_xg(1)
        nc.sync.dma_start(out=w8t[:, :, :, D:2 * D],
                          in_=w8r[:, :, :, D:2 * D])
        nc.sync.dma_start(out=eb_all[:],
                          in_=ebias.ap().rearrange("h p t -> p h t"))
        nc.sync.dma_start(out=pjb[:], in_=_bcast_ap(bproj.ap()[:], 128))
        nc.sync.dma_start(
            out=w_pj[:],
            in_=wproj.ap().rearrange("(c p) n -> p c n", p=128))
        make_identity(nc, ident[:])

        def qkv_steps(g):
            if g + 2 < G:
                _fetch_xg(g + 2)  # prefetch x two pairs ahead (3 buffers)
            xg = xg_tiles.pop(g)
            qkg = []
            qkg_map[g] = qkg

            def make_qk(m):
                def step():
                    ps = ps_big.tile([128, 512], F32, tag="big")
                    w_m = slice(m * 128, (m + 1) * 128)
                    for c in range(KC):
                        nc.tensor.matmul(ps[:, 0:PW],
                                         _dup2(w8t[:, c, 0, w_m]),
                                         xg[:, c, :, 0:PW],
                                         start=(c == 0),
                                         stop=(XSPLIT_QK and c == KC - 1),
                                         perf_mode=DR, skip_group_check=True)
                    if not XSPLIT_QK:
                        for p in range(KC // 2):
                            nc.tensor.matmul(ps[:, 0:PW],
                                             w8t[:, 2 * p:2 * p + 2, 1, w_m],
                                             xg[:, 2 * p:2 * p + 2, 0, 0:PW],
                                             start=False,
                                             stop=(p == KC // 2 - 1),
                                             perf_mode=DR,
                                             skip_group_check=True)
                    if m < 6:
                        # q: bf16 intermediate (pre-scaled by SQ8, bias in
                        # qkbt already carries SQ8) -> fp8 (hi, lo) pair.
                        # Pool cannot read PSUM and has no TensorScalar, so
                        # qf comes via DVE/ACT; hi is a convert, lo a sub.
                        scl = PS_SCL * SCALE * SQ8
                        qf = qfpool.tile([128, PW], BF16, tag="qf",
                                         name=f"qf{m}_{g}")
                        # qf mostly on DVE, hi/lo all on Pool; one qf per
                        # three m-groups goes to ACT to even the evac load
                        if m % 2 == 1:
                            nc.scalar.activation(qf[:], ps[:, 0:PW], IDENT,
                                                 bias=qkbt[:, m:m + 1],
                                                 scale=scl)
                        else:
                            nc.vector.tensor_scalar(qf[:], ps[:, 0:PW], scl,
                                                    qkbt[:, m:m + 1],
                                                    MUL, ADD)
                        q8 = qkpool.tile([128, 2, 400], FP8, tag=f"q8{m}",
                                         name=f"q8{m}_{g}")
                        nc.gpsimd.tensor_copy(q8[:, 0, 0:PW], qf[:])
                        nc.gpsimd.tensor_sub(q8[:, 1, 0:PW], qf[:],
                                             q8[:, 0, 0:PW])
                        qkg.append(q8)
                    else:
                        # k: single fp8 quant, bias dropped (softmax-inv.)
                        k8 = qkpool.tile([128, 400], FP8, tag=f"k8{m}",
                                         name=f"k8{m}_{g}")
                        if m % 2 == 0:
                            nc.scalar.activation(k8[:, 0:PW], ps[:, 0:PW],
                                                 IDENT, scale=PS_SCL * SK8)
                        else:
                            nc.vector.tensor_scalar(k8[:, 0:PW], ps[:, 0:PW],
                                                    PS_SCL * SK8, None, MUL)
                        qkg.append(k8)

                return step

            def make_v(j, gidx):
                def step():
                    img, it = j // 2, j % 2
                    gi = 2 * g + img
                    t0 = img * N + it * 128
                    tsz = 128 if it == 0 else 70  # even M for dual-fp8 LW
                    n0, nsz = (0, 512) if gidx == 0 else (512, 256)
                    ps = ps_v.tile([128, 512], F32, tag="pv")
                    wv = slice(2 * D + n0, 2 * D + n0 + nsz)
                    for c in range(KC):
                        nc.tensor.matmul(
                            ps[0:tsz, 0:nsz],
                            xg[:, c, :, t0:t0 + tsz],
                            _dup2(w8t[:, c, 0, wv]),
                            start=(c == 0), stop=False,
                            perf_mode=DR, skip_group_check=True)
                    for p in range(KC // 2):
                        nc.tensor.matmul(
                            ps[0:tsz, 0:nsz],
                            xg[:, 2 * p:2 * p + 2, 0, t0:t0 + tsz],
                            w8t[:, 2 * p:2 * p + 2, 1, wv],
                            start=False, stop=(p == KC // 2 - 1),
                            perf_mode=DR, skip_group_check=True)
                    if gidx == 0:
                        va = vapool.tile([128, H, HD + 1], BF16,
                                         tag=f"va{it}", name=f"va{it}_{gi}")
                        nc.gpsimd.memset(va[0:tsz, :, HD:HD + 1], 1.0)
                        if it == 0:
                            vmap[gi] = [va, None]
                        else:
                            vmap[gi][1] = va
                    va = vmap[gi][it]
                    hs = slice(0, 8) if gidx == 0 else (slice(8, 12))
                    # v bias is folded into the proj bias on the host, so
                    # the evac is a pure scale; alternate DVE/ACT.
                    ps3 = ps[0:tsz, 0:nsz].rearrange("p (h d) -> p h d",
                                                     d=HD)
                    if (j + gidx) % 2 == 0:
                        nc.vector.tensor_scalar(va[0:tsz, hs, 0:HD], ps3,
                                                PS_SCL, None, MUL)
                    else:
                        nc.scalar.activation(va[0:tsz, hs, 0:HD], ps3,
                                             IDENT, scale=PS_SCL)

                return step

            steps = [make_qk(m) for m in range(12)]
            # spread the 8 v half-tiles between qk m-groups to hide evac
            # latency; on pair 0 run them last — the v weights are still in
            # flight behind the q/k ones during the preload
            vsteps = [make_v(j, gx) for j in range(4) for gx in range(2)]
            if g == 0:
                steps.extend(vsteps)
            else:
                for i, vs in enumerate(reversed(vsteps)):
                    steps.insert(12 - i, vs)
            return steps

        # ---- attention steps for one pair ----
        def attn_steps(g):
            qkg = qkg_map.pop(g)
            steps = []
            pts = {}

            def score_step(img, h):
                def step():
                    if h == 0:
                        pts[img] = [
                            ptpool.tile([128, 6, PW], BF16, tag="pt0",
                                        name=f"pt0_{2 * g + img}"),
                            ptpool.tile([128, 6, PW], BF16, tag="pt1",
                                        name=f"pt1_{2 * g + img}")]
                    pt = pts[img][h // 6]
                    co = img * N
                    mq = h // 2
                    ro = (h % 2) * 64
                    mk = 6 + h // 2
                    q8 = qkg[mq]
                    k8 = qkg[mk]
                    ps = ps_sc.tile([128, PW], F32, tag="sc")
                    nc.tensor.matmul(ps[:, 0:N],
                                     _dup2(k8[ro:ro + 64, co:co + 128]),
                                     q8[ro:ro + 64, :, co:co + N],
                                     start=True, stop=True,
                                     perf_mode=DR, skip_group_check=True)
                    nc.tensor.matmul(ps[0:70, N:2 * N],
                                     _dup2(k8[ro:ro + 64, co + 128:co + 198]),
                                     q8[ro:ro + 64, :, co:co + N],
                                     start=True, stop=True,
                                     perf_mode=DR, skip_group_check=True)
                    hh = h % 6
                    nc.scalar.activation(pt[:, hh, :], ps[:], EXP,
                                         scale=E_SCL)
                    # Pool takes the FIRST head of each half (its ~877ns
                    # finishes before DVE's remaining five, so it never
                    # gates the AV that consumes the full pt half)
                    if hh == 0:
                        nc.gpsimd.tensor_mul(pt[:, hh, :], pt[:, hh, :],
                                             eb_all[:, h, :])
                    else:
                        nc.vector.tensor_mul(pt[:, hh, :], pt[:, hh, :],
                                             eb_all[:, h, :])

                return step

            def av_half(img, it, half):
                def step():
                    pt = pts[img][half]
                    va0, va1 = vmap[2 * g + img]
                    i0, isz = (0, 128) if it == 0 else (128, 69)
                    if half == 0:
                        ats[(img, it)] = atpool.tile(
                            [128, D], BF16, tag=f"at{it}",
                            name=f"at{it}_{g}_{img}")
                    at = ats[(img, it)]
                    av = ps_av.tile([128, 6 * 65], F32, tag="av")
                    for hh in range(6):
                        h = half * 6 + hh
                        nc.tensor.matmul(av[0:isz, hh * 65:(hh + 1) * 65],
                                         pt[:, hh, i0:i0 + isz],
                                         va0[:, h, :],
                                         start=True, stop=False)
                        nc.tensor.matmul(av[0:isz, hh * 65:(hh + 1) * 65],
                                         pt[0:69, hh, N + i0:N + i0 + isz],
                                         va1[0:69, h, :],
                                         start=False, stop=True)
                    av3 = av[0:isz].rearrange("p (h x) -> p h x", x=65)
                    rc = rcpool.tile([128, 6, 1], F32, tag="rc")
                    nc.vector.reciprocal(rc[0:isz], av3[:, :, 64:65])
                    nc.vector.tensor_mul(
                        at[0:isz, half * 384:(half + 1) * 384]
                        .rearrange("p (h x) -> p h x", x=HD),
                        av3[:, :, 0:HD],
                        _free_bcast(rc[0:isz], HD))

                return step

            def av_tr(img, it):
                def step():
                    at = ats[(img, it)]
                    gcol = g * PW + img * N
                    i0, isz = (0, 128) if it == 0 else (128, 69)
                    tcol = gcol + i0
                    # [128, 768] bf16 = 1536B fits a single psum bank
                    tp = ps_tr.tile([128, KC * 128], BF16, tag="tr")
                    for c in range(KC):
                        nc.tensor.transpose(tp[:, c * 128:c * 128 + isz],
                                            at[0:isz, c * 128:(c + 1) * 128],
                                            ident[0:isz, 0:isz])
                    nc.vector.tensor_copy(
                        attn_T[:, :, tcol:tcol + isz],
                        tp[:].rearrange("p (c t) -> p c t", t=128)
                        [:, :, 0:isz])

                return step

            ats = {}
            # interleave AV between the two half-head score batches so the
            # ACT exp chain (and the eb/normalize work) spreads across the
            # pair instead of clustering at its head
            for img in range(2):
                for h in range(6):
                    steps.append(score_step(img, h))
                if img == 1:
                    steps.append(av_half(0, 0, 1))
                    steps.append(av_half(0, 1, 1))
                    steps.append(av_tr(0, 0))
                    steps.append(av_tr(0, 1))
                for h in range(6, H):
                    steps.append(score_step(img, h))
                steps.append(av_half(img, 0, 0))
                steps.append(av_half(img, 1, 0))
            steps.append(av_half(1, 0, 1))
            steps.append(av_half(1, 1, 1))
            steps.append(av_tr(1, 0))
            steps.append(av_tr(1, 1))
            return steps

        # ---- main loop: interleave qkv(g) with attention(g-1) ----
        pending = []

        def drain(k):
            for _ in range(k):
                if pending:
                    pending.pop(0)()

        for g in range(G):
            qs = qkv_steps(g)
            n_q = len(qs)
            n_a = len(pending)
            for i, q in enumerate(qs):
                want = ((i + 1) * n_a) // n_q
                done = n_a - len(pending)
                drain(want - done)
                q()
            drain(len(pending))
            pending = attn_steps(g)

        # ---- output projection interleaved with last pair's attention ----
        def proj_step(t0):
            def step():
                sz = min(128, T - t0)
                ot = opool.tile([128, D], F32, tag="osb")
                for (n0, nsz) in ((0, 512), (512, 256)):
                    ps = ps_big.tile([128, 512], F32, tag="big")
                    for c in range(KC):
                        nc.tensor.matmul(ps[0:sz, 0:nsz],
                                         attn_T[:, c, t0:t0 + sz],
                                         w_pj[:, c, n0:n0 + nsz],
                                         start=(c == 0), stop=(c == KC - 1))
                    nc.vector.tensor_add(ot[0:sz, n0:n0 + nsz],
                                         ps[0:sz, 0:nsz],
                                         pjb[0:sz, n0:n0 + nsz])
                    nc.sync.dma_start(out=out.ap()[t0:t0 + sz, n0:n0 + nsz],
                                      in_=ot[0:sz, n0:n0 + nsz])

            return step

        safe_t = (G - 1) * PW
        proj_tiles = list(range(0, T, 128))
        early = [t for t in proj_tiles if t + 128 <= safe_t]
        n_a = len(pending)
        n_p = len(early)
        assert n_a == 36  # late-tile drain points below index this layout

        def drain_until(k):
            drain(k - (n_a - len(pending)))

        for i, t0 in enumerate(early):
            proj_step(t0)()
            want = ((i + 1) * n_a) // max(n_p, 1)
            done = n_a - len(pending)
            drain(want - done)
        # late tiles as soon as their attn_T columns are transposed:
        # step 23 = av_tr(img0, it1), 34 = av_tr(img1, it0), 35 = the rest
        drain_until(24)
        proj_step(2688)()
        proj_step(2816)()
        drain_until(35)
        proj_step(2944)()
        drain(len(pending))
        proj_step(3072)()

    nc.compile()
    return nc


def _get_graph():
    global _GRAPH
    if _GRAPH is None:
        _GRAPH = _build()
    return _GRAPH


def kernel(x, qkv_w, qkv_b, proj_w, proj_b, rel_bias_table, rel_index):
    global LAST_EXEC_NS
    FP8NP = ml_dtypes.float8_e4m3
    x = np.asarray(x, dtype=np.float32)
    qkv_w = np.asarray(qkv_w, dtype=np.float32)
    qkv_b = np.asarray(qkv_b, dtype=np.float32)
    proj_w = np.asarray(proj_w, dtype=np.float32)
    proj_b = np.asarray(proj_b, dtype=np.float32)
    rel_bias_table = np.asarray(rel_bias_table, dtype=np.float32)
    rel_index = np.asarray(rel_index)

    # qkv weights: fp8 hi/lo split at scale SW, [D, 2, 3D]
    wT = np.ascontiguousarray(qkv_w.T) * SW
    wh = wT.astype(FP8NP)
    wl = (wT - wh.astype(np.float32)).astype(FP8NP)
    # [128, KC, 2, 3D]: partition-major so the per-m-col DMA merges to 3 dims
    w8 = np.stack([wh, wl], axis=1)               # [D, 2, 3D]
    w8 = w8.reshape(KC, 128, 2, 3 * D).transpose(1, 0, 2, 3)
    w8 = np.ascontiguousarray(w8)
    # per-m-group bias columns for q [128, 12]; q groups carry the score
    # scale AND the fp8 quantization scale SQ8 (qf is pre-scaled).
    # k bias is dropped on device (softmax-invariant), cols 6-11 unused.
    qkb = np.empty((128, 12), dtype=np.float32)
    for m in range(12):
        col = qkv_b[m * 128:(m + 1) * 128]
        qkb[:, m] = col * SCALE * SQ8 if m < 6 else col
    vbias = qkv_b[2 * D:3 * D]
    wprojT = np.ascontiguousarray(proj_w.T).astype(ml_dtypes.bfloat16)
    # v bias folded into the projection bias: out = (at + bv) @ WpT + bp
    pjb_eff = proj_b + vbias @ wprojT.astype(np.float32)
    # dense rel-pos bias -> [h, j(key), i(query)], exponentiated, packed into
    # the [128, 394] two-key-tile layout (rows 70:128 of cols 197:394 unused)
    bias = rel_bias_table[rel_index]  # [N, N, H]
    biasTh = np.transpose(bias, (2, 1, 0)).astype(np.float32)
    ebias = np.ones((H, 128, PW), dtype=np.float32)
    ebias[:, 0:128, 0:N] = np.exp(biasTh[:, 0:128, :])
    ebias[:, 0:69, N:PW] = np.exp(biasTh[:, 128:N, :])
    ebias = ebias.astype(ml_dtypes.bfloat16)

    nc = _get_graph()
    in_maps = []
    for i in range(NCORES):
        xs = x[i * BL:(i + 1) * BL].reshape(T, D)
        xT = np.ascontiguousarray(xs.T) * SX
        xh = xT.astype(FP8NP)
        xl = (xT - xh.astype(np.float32)).astype(FP8NP)
        # [G, 128, KC, 2, 400]: partition-major, token dim zero-padded to
        # 400 so each per-pair fetch is one contiguous 4800B run/partition
        xhl = np.stack([xh, xl], axis=1)          # [D, 2, T]
        xhl = xhl.reshape(KC, 128, 2, G, PW).transpose(3, 1, 0, 2, 4)
        xpad = np.zeros((G, 128, KC, 2, 400), dtype=FP8NP)
        xpad[:, :, :, :, 0:PW] = xhl
        in_maps.append({
            "xhl": xpad,
            "w8": w8,
            "qkb": qkb,
            "wproj": wprojT,
            "bproj": pjb_eff,
            "ebias": ebias,
        })
    res = run_bass_kernel_spmd(nc, in_maps, core_ids=list(range(NCORES)))
    LAST_EXEC_NS = res.exec_time_ns
    outs = [np.asarray(res.results[i]["out"], dtype=np.float32)
            for i in range(NCORES)]
    return np.concatenate([o.reshape(BL, N, D) for o in outs], axis=0)
